# revision 12
# baseline (speedup 1.0000x reference)
"""Trainium2 Bass kernel for nn_Block1_87144886436577 (vq_codebook).

v2: same math as v1 (analytic jacobian collapse -> tap matmuls,
9-candidate argmin with tie rule, masked scatter via tap matmuls), but
the data plumbing is rebuilt for latency: no DRAM scratch round trips
(the per-o patch image and the sel image are staged on single SBUF
partitions and fanned out with overlapping-window SBUF->SBUF DMAs),
host constants are repacked so each loads with one contiguous DMA,
the eight x-side gathers run as one GPSIMD indirect_copy, and DMAs are
split across the two HWDGE queues (sync + scalar).

Single-core program; all 8 cores run identical replicas. Output read
from core 0.
"""
import sys

import numpy as np

for _p in ("/opt/trn_rl_repo",):
    if _p not in sys.path:
        sys.path.insert(0, _p)

import concourse.bass as bass
import concourse.mybir as mybir
import concourse.tile as tile

F32 = mybir.dt.float32
U16 = mybir.dt.uint16
AF = mybir.ActivationFunctionType
ALU = mybir.AluOpType
AX = mybir.AxisListType
AP = bass.AP

N_CORES = 8


def v(t, off, pat):
    """Custom-view AP over a tile (t = AP returned by pool.tile)."""
    return AP(t.tensor, t.offset + off, pat)


def _e(r):
    return 1 if r >= 1 else 0


def _consts():
    """Host-precomputed constant tensors (input-independent).

    cstf [128, 1440] f32: ident | oidx128 | ematT | oidx9 | zc
    cstu [128, 96] u16:   idxX-T | idxS-T | idxE | pad
    """
    ident128 = np.eye(128, dtype=np.float32)
    oidx128 = np.tile((np.arange(128) % 64).astype(np.float32)[None, :],
                      (128, 1))

    # xsel gather index streams (same values as v1, host layout [128, 64]:
    # col t*8 + j//16, row 16g + j%16 for tile t, stream j).
    idxX = np.zeros((128, 64), np.uint16)
    for t in range(8):
        k2y, k2xh = t // 2, t % 2
        for g in range(8):
            k2xp = g // 4
            k1y = g % 4
            k2x = 2 * k2xh + k2xp
            for j in range(128):
                b, oy, ox = j // 64, (j % 64) // 8, j % 8
                idxX[16 * g + j % 16, t * 8 + j // 16] = (
                    b * 1444 + (4 * oy + 2 * k2y + k1y) * 38
                    + 4 * ox + 2 * k2x)

    # sel gather (2 tiles s, packed into [128, 16])
    idxS = np.zeros((128, 16), np.uint16)
    for s in range(2):
        for g in range(8):
            k2yp, k2x = g // 4, g % 4
            k2y = s * 2 + k2yp
            for j in range(128):
                b, oy, ox = j // 64, (j % 64) // 8, j % 8
                idxS[16 * g + j % 16, s * 8 + j // 16] = (
                    b * 1444 + (4 * oy + 2 * k2y) * 38 + 4 * ox + 2 * k2x)

    # expansion matrices, transposed host layout [128, 8*128]
    emat = np.zeros((8, 128, 128), np.float32)
    for t in range(8):
        k2y, k2xh = t // 2, t % 2
        for p in range(128):
            k2xp, k1y, k1x = p // 64, (p % 64) // 16, p % 4
            k2x = 2 * k2xh + k2xp
            r = (k2y % 2) * 64 + k2x * 16 + k1y * 4 + k1x
            emat[t, r, p] = 1.0
    ematT = np.transpose(emat, (1, 0, 2)).reshape(128, 1024)

    # E9 gather
    idxE = np.zeros((128, 9), np.uint16)
    for r in range(4):
        for h in range(2):
            g = r * 2 + h
            for j in range(144):
                ixl, jj = j // 9, j % 9
                jy, jx = jj // 3, jj % 3
                t_ = ixl % 4
                s = 4 * h + ixl // 4
                dy = r - 4 * _e(r) + 4 * jy + 3
                dx = t_ - 4 * _e(t_) + 4 * jx + 3
                oxp = s + _e(t_) - jx + 2
                if 0 <= dy < 10 and 0 <= dx < 10:
                    idx = (2 - jy) * 1200 + oxp * 100 + dy * 10 + dx
                else:
                    idx = 0  # guaranteed-zero pad cell
                idxE[16 * g + j % 16, j // 16] = idx

    # candidate o-index (3000 = invalid) and first-noncovering-o tables
    oidx9 = np.full((128, 144), 3000.0, np.float32)
    zc = np.zeros((128, 16), np.float32)
    for r in range(4):
        for h in range(2):
            for b in range(2):
                for q in range(8):
                    p = r * 32 + h * 16 + b * 8 + q
                    iy = 4 * q + r
                    for ixl in range(16):
                        ix = 16 * h + ixl
                        t_ = ix % 4
                        s = ix // 4
                        for jj in range(9):
                            jy, jx = jj // 3, jj % 3
                            oy = q + _e(r) - jy
                            ox = s + _e(t_) - jx
                            dy = iy - 4 * oy + 3
                            dx = ix - 4 * ox + 3
                            if (0 <= oy < 8 and 0 <= ox < 8
                                    and 0 <= dy < 10 and 0 <= dx < 10):
                                oidx9[p, ixl * 9 + jj] = oy * 8 + ox
                        for o in range(64):
                            oy, ox = o // 8, o % 8
                            if not (0 <= iy - 4 * oy + 3 < 10
                                    and 0 <= ix - 4 * ox + 3 < 10):
                                zc[p, ixl] = float(o)
                                break
    cstf = np.concatenate(
        [ident128, oidx128, ematT, oidx9, zc], axis=1)
    assert cstf.shape == (128, 1440), cstf.shape
    cstu = np.concatenate(
        [idxX, idxS, idxE, np.zeros((128, 7), np.uint16)], axis=1)
    assert cstu.shape == (128, 96), cstu.shape
    return {"cstf": np.ascontiguousarray(cstf),
            "cstu": np.ascontiguousarray(cstu)}


def build_program(nc):
    x_d = nc.declare_dram_parameter("x", [2, 3, 32, 32], F32, isOutput=False)
    w1_d = nc.declare_dram_parameter("w1", [32, 3, 4, 4], F32, isOutput=False)
    b1_d = nc.declare_dram_parameter("b1", [32], F32, isOutput=False)
    w2_d = nc.declare_dram_parameter("w2", [64, 32, 4, 4], F32, isOutput=False)
    b2_d = nc.declare_dram_parameter("b2", [64], F32, isOutput=False)
    k_d = nc.declare_dram_parameter("K", [512, 64], F32, isOutput=False)
    v_d = nc.declare_dram_parameter("V", [512, 64], F32, isOutput=False)
    cf_d = nc.declare_dram_parameter("cstf", [128, 1440], F32, isOutput=False)
    cu_d = nc.declare_dram_parameter("cstu", [128, 96], U16, isOutput=False)
    out_d = nc.declare_dram_parameter("out", [2, 64, 8, 8], F32,
                                      isOutput=True)

    with tile.TileContext(nc) as tc:
        with (
            tc.tile_pool(name="const", bufs=1) as cpool,
            tc.tile_pool(name="work", bufs=1) as wpool,
            tc.tile_pool(name="psA", bufs=2, space="PSUM") as psA,
            tc.tile_pool(name="psB", bufs=2, space="PSUM") as psB,
            tc.tile_pool(name="psC", bufs=1, space="PSUM") as psC,
            tc.tile_pool(name="dram", bufs=1, space="DRAM") as dpool,
        ):
            dmaS = nc.sync.dma_start
            dmaA = nc.scalar.dma_start

            # ---- phase-A-critical loads first ----
            xp2 = wpool.tile([3, 2892], F32)  # [ci, (b, 38x38)] + 4 tail
            nc.vector.memset(xp2[:], 0.0)
            xpitch = xp2.ap[0][0]
            for b in range(2):
                (dmaS if b == 0 else dmaA)(
                    v(xp2, b * 1444 + 117,
                      [[xpitch, 3], [38, 32], [1, 32]]),
                    AP(x_d, b * 3072, [[1024, 3], [32, 32], [1, 32]]))

            w1taps = wpool.tile([3, 512], F32)        # [ci, (m,k1)]
            dmaS(w1taps[:], AP(w1_d, 0, [[16, 3], [48, 32], [1, 16]]))
            b1t = wpool.tile([32, 1], F32)
            dmaA(b1t[:], AP(b1_d, 0, [[1, 32], [1, 1]]))
            w1sb = wpool.tile([32, 48], F32)          # [m, (ci,k1)]
            dmaA(w1sb[:], AP(w1_d, 0, [[48, 32], [1, 48]]))

            # conv2 / hopfield staging (needed a few us later)
            w2sb = wpool.tile([32, 1024], F32)        # [m, (c,k2)]
            dmaS(w2sb[:], AP(w2_d, 0, [[16, 32], [512, 64], [1, 16]]))
            w2c2 = wpool.tile([64, 512], F32)         # [c, (m,k2)]
            dmaA(w2c2[:], AP(w2_d, 0, [[512, 64], [16, 32], [1, 16]]))
            b2t = wpool.tile([64, 1], F32)
            dmaA(b2t[:], AP(b2_d, 0, [[1, 64], [1, 1]]))

            cst = cpool.tile([128, 1440], F32)
            dmaS(cst[:], cf_d[:])
            ident = cst[:, 0:128]
            oidx9 = cst[:, 1280:1424]
            zct = cst[:, 1424:1440]

            def emt_t(t):
                return cst[:, 256 + t * 128:256 + (t + 1) * 128]
            cstu = cpool.tile([128, 96], U16)
            dmaA(cstu[:], cu_d[:])
            ixX = cstu[:, 0:64]
            ixS = cstu[:, 64:80]
            ixE = cstu[:, 80:89]

            ktl = wpool.tile([128, 256], F32)  # K 4 col-tiles
            dmaS(ktl[:], AP(k_d, 0, [[64, 128], [8192, 4], [1, 64]]))
            v_sb = wpool.tile([128, 256], F32)
            dmaA(v_sb[:], AP(v_d, 0, [[64, 128], [8192, 4], [1, 64]]))

            # ---- Phase A: conv1 ----
            y1ps = psA.tile([32, 512], F32, tag="psA", name="y1ps")
            for k in range(16):
                k1y, k1x = k // 4, k % 4
                nc.tensor.matmul(
                    y1ps[:],
                    v(w1taps, 4 * k1y + k1x,
                      [[w1taps.ap[0][0], 3], [16, 32]]),
                    v(xp2, 78 + 38 * k1y + k1x,
                      [[xpitch, 3], [1444, 2], [76, 16], [2, 16]]),
                    start=(k == 0), stop=(k == 15))
            y1sb = wpool.tile([32, 512], F32)  # [m, (b,py,px)]
            nc.scalar.activation(y1sb[:], y1ps[:], AF.Relu, bias=b1t[:])

            # ---- x-side staging (overlaps A/B; off critical path) ----
            # w1sb2p / w1fp build
            w1sb2 = wpool.tile([32, 48], F32)   # [m, (k1x,ci,k1y)]
            dmaA(w1sb2[:], AP(w1_d, 0, [[48, 32], [16, 3], [1, 16]]))
            w1sb2p = wpool.tile([32, 64], F32)  # [m, (k1y,ci4,k1x)]
            nc.vector.memset(w1sb2p[:], 0.0)
            nc.vector.tensor_copy(
                v(w1sb2p, 0, [[w1sb2p.ap[0][0], 32], [16, 4], [4, 3], [1, 4]]),
                v(w1sb2, 0, [[w1sb2.ap[0][0], 32], [4, 4], [16, 3], [1, 4]]))
            w1fp_ps = psB.tile([64, 32], F32, tag="psB", name="w1fp_ps")
            nc.tensor.transpose(w1fp_ps[:], w1sb2p[:], cst[0:32, 0:32])
            w1fp = wpool.tile([128, 32], F32)
            nc.scalar.copy(w1fp[0:64, :], w1fp_ps[:])
            dmaS(w1fp[64:128, :], w1fp[0:64, :])

            # data_x: 12 base rows via one overlapping-window fanout DMA
            data_x = wpool.tile([128, 2888], F32)
            nc.vector.memset(data_x[0:16, :], 0.0)
            dmaS(data_x[0:12, :],
                 v(xp2, 0, [[xpitch, 3], [1, 4], [1, 2888]]))
            for d in (16, 32, 64):
                dmaA(data_x[d:2 * d, :], data_x[0:d, :])
            xgall = wpool.tile([128, 1024], F32)
            nc.gpsimd.indirect_copy(
                v(xgall, 0, [[xgall.ap[0][0], 128], [1, 1024], [1, 1]]),
                data_x[:], ixX, True)

            # K^T via 4 PE transposes
            kt_sb = wpool.tile([64, 512], F32)
            for t in range(4):
                kt_ps = psB.tile([64, 128], F32, tag="psB", name="kt_ps")
                nc.tensor.transpose(kt_ps[:], ktl[:, t * 64:(t + 1) * 64],
                                    ident)
                nc.scalar.copy(kt_sb[:, t * 128:(t + 1) * 128], kt_ps[:])

            # early staging for the argmin/sel fanouts
            z225 = cpool.tile([128, 225], F32)
            nc.vector.memset(z225[:], 0.0)
            zneg = cpool.tile([128, 95], F32)
            nc.vector.memset(zneg[:], -1.0)
            ed4ps = wpool.tile([2, 14400], F32)  # [b, (12 oyp,12 oxp,100)]
            ep_p = ed4ps.ap[0][0]
            dmaS(v(ed4ps, 0, [[ep_p, 2], [225, 64], [1, 225]]), z225[:])
            sel_pad = dpool.tile([3040], F32)
            dmaA(v(sel_pad, 0, [[95, 32], [1, 95]]), zneg[0:32, :])

            # ---- Phase A: conv2 ----
            y1p = wpool.tile([32, 648], F32)   # [m, (b,18,18)] padded
            nc.vector.memset(y1p[:], 0.0)
            ypitch = y1p.ap[0][0]
            nc.vector.tensor_copy(
                v(y1p, 19, [[ypitch, 32], [324, 2], [18, 16], [1, 16]]),
                v(y1sb, 0,
                  [[y1sb.ap[0][0], 32], [256, 2], [16, 16], [1, 16]]))
            m1p = wpool.tile([32, 648], F32)
            nc.vector.tensor_scalar(m1p[:], y1p[:], 0.0, None, ALU.is_gt)

            def tapview(tl, k2, pitch):
                k2y, k2x = k2 // 4, k2 % 4
                return v(tl, 18 * k2y + k2x,
                         [[pitch, 32], [324, 2], [36, 8], [2, 8]])

            ypre = psA.tile([64, 128], F32, tag="psA", name="ypre")
            for k2 in range(16):
                nc.tensor.matmul(
                    ypre[:],
                    v(w2sb, k2, [[w2sb.ap[0][0], 32], [16, 64]]),
                    tapview(y1p, k2, ypitch),
                    start=(k2 == 0), stop=(k2 == 15))
            yT = wpool.tile([64, 128], F32)    # [c, (b,o)]
            nc.scalar.activation(yT[:], ypre[:], AF.Relu, bias=b2t[:])
            m2T = wpool.tile([64, 128], F32)
            nc.vector.tensor_scalar(m2T[:], yT[:], 0.0, None, ALU.is_gt)

            def hopfield(src, tag):
                """src [c 64, (b,o) 128] SBUF -> yq [(b,o) 128, c 64] SBUF."""
                a_ps = psA.tile([128, 512], F32, tag="psA",
                                name=f"a_ps{tag}")
                nc.tensor.matmul(a_ps[:], src, kt_sb[:], start=True,
                                 stop=True)
                rmax = wpool.tile([128, 1], F32, name=f"rmax{tag}")
                nc.vector.tensor_reduce(rmax[:], a_ps[:], AX.X, ALU.max)
                negbm = wpool.tile([128, 1], F32, name=f"negbm{tag}")
                nc.vector.tensor_scalar(negbm[:], rmax[:], -0.125, None,
                                        ALU.mult)
                p_sb = wpool.tile([128, 512], F32, name=f"p_sb{tag}")
                ssum = wpool.tile([128, 1], F32, name=f"ssum{tag}")
                nc.scalar.activation(p_sb[:], a_ps[:], AF.Exp, bias=negbm[:],
                                     scale=0.125, accum_out=ssum[:])
                rec = wpool.tile([128, 1], F32, name=f"rec{tag}")
                nc.vector.reciprocal(rec[:], ssum[:])
                nc.vector.tensor_scalar(p_sb[:], p_sb[:], rec[:], None,
                                        ALU.mult)
                yq_ps = psC.tile([128, 64], F32, tag="psC",
                                 name=f"yq_ps{tag}")
                for t in range(4):
                    pt_ps = psB.tile([128, 128], F32, tag="psB",
                                     name=f"pt_ps{tag}{t}")
                    nc.tensor.transpose(pt_ps[:],
                                        p_sb[:, t * 128:(t + 1) * 128],
                                        ident)
                    pt_sb = wpool.tile([128, 128], F32, tag="pt_sb",
                                       name=f"pt_sb{tag}{t}")
                    nc.scalar.copy(pt_sb[:], pt_ps[:])
                    nc.tensor.matmul(yq_ps[:], pt_sb[:],
                                     v_sb[:, t * 64:(t + 1) * 64],
                                     start=(t == 0), stop=(t == 3))
                yq_sb = wpool.tile([128, 64], F32, name=f"yq_sb{tag}")
                nc.scalar.copy(yq_sb[:], yq_ps[:])
                return yq_sb

            yq1 = hopfield(yT[:], "h1")

            yqT_ps = psB.tile([64, 128], F32, tag="psB", name="yqT_ps")
            nc.tensor.transpose(yqT_ps[:], yq1[:], ident)
            r2T = wpool.tile([64, 128], F32)
            nc.vector.scalar_tensor_tensor(r2T[:], yqT_ps[:], -1.0, yT[:],
                                           ALU.mult, ALU.add)
            nc.vector.tensor_mul(r2T[:], r2T[:], m2T[:])

            # ---- Phase B: e_patch + argmin ----
            w1s16 = wpool.tile([32, 16], F32)
            nc.vector.tensor_reduce(
                w1s16[:],
                v(w1sb, 0, [[w1sb.ap[0][0], 32], [1, 16], [16, 3]]),
                AX.X, ALU.add)
            w1si = wpool.tile([32, 256], F32)  # w1s 16x16 zero-pad image
            nc.vector.memset(w1si[:], 0.0)
            nc.vector.tensor_copy(
                v(w1si, 102, [[w1si.ap[0][0], 32], [16, 4], [1, 4]]),
                w1s16[:])
            w1stp = wpool.tile([32, 1600], F32)
            for k2y in range(4):
                nc.vector.tensor_copy(
                    v(w1stp, k2y * 400,
                      [[w1stp.ap[0][0], 32], [100, 4], [10, 10], [1, 10]]),
                    v(w1si, (6 - 2 * k2y) * 16 + 6,
                      [[w1si.ap[0][0], 32], [-2, 4], [16, 10], [1, 10]]))

            g1m = [wpool.tile([32, 128], F32, name=f"g1m{k2}")
                   for k2 in range(16)]
            for k2 in range(16):
                g1ps = psB.tile([32, 128], F32, tag="psB", name=f"g1ps{k2}")
                nc.tensor.matmul(
                    g1ps[:],
                    v(w2c2, k2, [[w2c2.ap[0][0], 64], [16, 32]]),
                    r2T[:], start=True, stop=True)
                nc.vector.tensor_tensor(g1m[k2][:], g1ps[:],
                                        tapview(m1p, k2, ypitch), ALU.mult)

            ep_ps = psA.tile([100, 128], F32, tag="psA", name="ep_ps")
            for k2 in range(16):
                nc.tensor.matmul(
                    ep_ps[:],
                    w1stp[:, k2 * 100:(k2 + 1) * 100],
                    g1m[k2][:], start=(k2 == 0), stop=(k2 == 15))
            ep_sb = wpool.tile([100, 128], F32)
            nc.scalar.copy(ep_sb[:], ep_ps[:])
            ep2_ps = psB.tile([128, 100], F32, tag="psB", name="ep2_ps")
            nc.tensor.transpose(ep2_ps[:], ep_sb[:], cst[0:100, 0:100])
            ep2 = wpool.tile([128, 100], F32)  # [(b,oy,ox), (dy,dx)]
            nc.scalar.copy(ep2[:], ep2_ps[:])

            # dense hop then scatter into the per-b padded image (SBUF)
            ed4x = wpool.tile([2, 6400], F32)
            e4xp = ed4x.ap[0][0]
            dmaS(ed4x[:], ep2[:])
            dmaS(v(ed4ps, 2 * 1200 + 2 * 100,
                   [[ep_p, 2], [1200, 8], [1, 800]]),
                 v(ed4x, 0, [[e4xp, 2], [800, 8], [1, 800]]))

            # data_e: 8 fanout DMAs (one per (r,h) group)
            data_e = wpool.tile([128, 3600], F32)
            dp = data_e.ap[0][0]
            for r in range(4):
                for h in range(2):
                    g = r * 2 + h
                    (dmaS if h == 0 else dmaA)(
                        data_e[g * 16:(g + 1) * 16, :],
                        v(ed4ps, _e(r) * 1200,
                          [[ep_p, 2], [1200, 8], [1, 3600]]))
            e9 = wpool.tile([128, 144], F32)
            e9pitch = e9.ap[0][0]
            nc.gpsimd.indirect_copy(
                v(e9, 0, [[e9pitch, 128], [1, 144], [1, 1]]),
                data_e[:], ixE, True)

            # argmin with reference tie semantics
            mincand = wpool.tile([128, 16], F32)
            nc.vector.tensor_reduce(
                mincand[:], v(e9, 0, [[e9pitch, 128], [9, 16], [1, 9]]),
                AX.X, ALU.min)
            mstar = wpool.tile([128, 16], F32)
            nc.vector.tensor_scalar(mstar[:], mincand[:], 0.0, None, ALU.min)
            eq9 = wpool.tile([128, 144], F32)
            nc.vector.tensor_tensor(
                v(eq9, 0, [[eq9.ap[0][0], 128], [9, 16], [1, 9]]),
                v(e9, 0, [[e9pitch, 128], [9, 16], [1, 9]]),
                v(mstar, 0, [[mstar.ap[0][0], 128], [1, 16], [0, 9]]),
                ALU.is_equal)
            cs = wpool.tile([128, 144], F32)
            nc.vector.scalar_tensor_tensor(cs[:], eq9[:], -1000.0, oidx9,
                                           ALU.mult, ALU.add)
            minc2 = wpool.tile([128, 16], F32)
            nc.vector.tensor_reduce(
                minc2[:], v(cs, 0, [[cs.ap[0][0], 128], [9, 16], [1, 9]]),
                AX.X, ALU.min)
            zeq = wpool.tile([128, 16], F32)
            nc.vector.tensor_scalar(zeq[:], mstar[:], 0.0, None,
                                    ALU.is_equal)
            zsc = wpool.tile([128, 16], F32)
            nc.vector.scalar_tensor_tensor(zsc[:], zeq[:], -1000.0, zct,
                                           ALU.mult, ALU.add)
            sel16 = wpool.tile([128, 16], F32)
            nc.vector.tensor_tensor(sel16[:], minc2[:], zsc[:], ALU.min)
            nc.vector.tensor_scalar(sel16[:], sel16[:], 1000.0, None,
                                    ALU.add)

            # sel -> DRAM padded image (pad = -1, pre-filled)
            for r in range(4):
                for h in range(2):
                    (dmaS if h == 0 else dmaA)(
                        v(sel_pad, 117 + 38 * r + 16 * h,
                          [[1444, 2], [152, 8], [1, 16]]),
                        sel16[r * 32 + h * 16:r * 32 + h * 16 + 16, :])

            # ---- Phase C: sel gather + mask expansion ----
            # data_s: 8 fanout DMAs (16 shifted rows each, one per replica)
            data_s = wpool.tile([128, 2888], F32)
            for g in range(8):
                (dmaS if g % 2 == 0 else dmaA)(
                    data_s[g * 16:(g + 1) * 16, :],
                    v(sel_pad, 0, [[38, 4], [1, 4], [1, 2888]]))
            sg = wpool.tile([128, 256], F32)
            nc.gpsimd.indirect_copy(
                v(sg, 0, [[sg.ap[0][0], 128], [1, 256], [1, 1]]),
                data_s[:], ixS, True)
            nc.vector.tensor_tensor(
                v(sg, 0, [[sg.ap[0][0], 128], [128, 2], [1, 128]]),
                v(sg, 0, [[sg.ap[0][0], 128], [128, 2], [1, 128]]),
                v(cst, 128, [[cst.ap[0][0], 128], [0, 2], [1, 128]]),
                ALU.is_equal)
            selm2 = [sg[:, 0:128], sg[:, 128:256]]

            xsel = []
            for t in range(8):
                mx_ps = psB.tile([128, 128], F32, tag="psB", name=f"mx{t}")
                nc.tensor.matmul(mx_ps[:], emt_t(t),
                                 selm2[(t // 2) // 2],
                                 start=True, stop=True)
                xs = wpool.tile([128, 128], F32, name=f"xs{t}")
                nc.vector.tensor_tensor(xs[:],
                                        xgall[:, t * 128:(t + 1) * 128],
                                        mx_ps[:], ALU.mult)
                xsel.append(xs)

            zm = [wpool.tile([32, 128], F32, name=f"zm{k2}")
                  for k2 in range(16)]
            for k2 in range(16):
                k2y, k2x = k2 // 4, k2 % 4
                t = k2y * 2 + k2x // 2
                half = (k2x % 2) * 64
                z_ps = psB.tile([32, 128], F32, tag="psB", name=f"z_ps{k2}")
                nc.tensor.matmul(z_ps[:], w1fp[half:half + 64, :],
                                 xsel[t][half:half + 64, :],
                                 start=True, stop=True)
                nc.vector.tensor_tensor(zm[k2][:], z_ps[:],
                                        tapview(m1p, k2, ypitch), ALU.mult)

            ym_ps = psA.tile([128, 64], F32, tag="psA", name="ym_ps")
            for k2 in range(16):
                nc.tensor.matmul(
                    ym_ps[:], zm[k2][:],
                    v(w2sb, k2, [[w2sb.ap[0][0], 32], [16, 64]]),
                    start=(k2 == 0), stop=(k2 == 15))

            yTT_ps = psB.tile([128, 64], F32, tag="psB", name="yTT_ps")
            nc.tensor.transpose(yTT_ps[:], yT[:], cst[0:64, 0:64])
            m2g = wpool.tile([128, 64], F32)
            nc.vector.tensor_scalar(m2g[:], yTT_ps[:], 0.0, None, ALU.is_gt)
            ymm = wpool.tile([128, 64], F32)
            nc.vector.tensor_tensor(ymm[:], ym_ps[:], m2g[:], ALU.mult)

            t2_ps = psB.tile([64, 128], F32, tag="psB", name="t2_ps")
            nc.tensor.transpose(t2_ps[:], ymm[:], ident)
            ymmT = wpool.tile([64, 128], F32)
            nc.scalar.copy(ymmT[:], t2_ps[:])

            yq2 = hopfield(ymmT[:], "h2")

            tr_ps = psB.tile([64, 128], F32, tag="psB", name="tr_ps")
            nc.tensor.transpose(tr_ps[:], yq2[:], ident)
            outT = wpool.tile([64, 128], F32)
            nc.scalar.copy(outT[:], tr_ps[:])
            for b in range(2):
                (dmaS if b == 0 else dmaA)(
                    AP(out_d, b * 4096, [[64, 64], [8, 8], [1, 8]]),
                    outT[:, b * 64:(b + 1) * 64])

    return nc


_CACHE = {}


def kernel(**inputs) -> np.ndarray:
    from concourse.bass_utils import run_bass_kernel_spmd
    if "nc" not in _CACHE:
        from concourse import bacc
        nc = bacc.Bacc("TRN2", target_bir_lowering=False, debug=False,
                       num_devices=N_CORES)
        build_program(nc)
        nc.compile()
        _CACHE["nc"] = nc
        _CACHE["consts"] = _consts()
    nc = _CACHE["nc"]
    feed = {}
    for k, val in inputs.items():
        arr = np.asarray(val)
        if arr.dtype != np.uint16:
            arr = np.asarray(arr, np.float32)
        feed[k] = np.ascontiguousarray(arr)
    for k, val in _CACHE["consts"].items():
        feed[k] = val
    in_maps = [dict(feed) for _ in range(N_CORES)]
    res = run_bass_kernel_spmd(nc, in_maps, list(range(N_CORES)))
    return np.asarray(res.results[0]["out"], np.float32)


# revision 32
# speedup vs baseline: 1.3805x; 1.3805x over previous
"""Trainium2 Bass kernel for nn_Block1_87144886436577 (vq_codebook).

v2: same math as v1 (analytic jacobian collapse -> tap matmuls,
9-candidate argmin with tie rule, masked scatter via tap matmuls), but
the data plumbing is rebuilt for latency: no DRAM scratch round trips
(the per-o patch image and the sel image are staged on single SBUF
partitions and fanned out with overlapping-window SBUF->SBUF DMAs),
host constants are repacked so each loads with one contiguous DMA,
the eight x-side gathers run as one GPSIMD indirect_copy, and DMAs are
split across the two HWDGE queues (sync + scalar).

Single-core program; all 8 cores run identical replicas. Output read
from core 0.
"""
import sys

import numpy as np

for _p in ("/opt/trn_rl_repo",):
    if _p not in sys.path:
        sys.path.insert(0, _p)

import concourse.bass as bass
import concourse.mybir as mybir
import concourse.tile as tile

F32 = mybir.dt.float32
BF16 = mybir.dt.bfloat16
U16 = mybir.dt.uint16
AF = mybir.ActivationFunctionType
ALU = mybir.AluOpType
AX = mybir.AxisListType
AP = bass.AP

N_CORES = 8


def v(t, off, pat):
    """Custom-view AP over a tile (t = AP returned by pool.tile)."""
    return AP(t.tensor, t.offset + off, pat)


def _e(r):
    return 1 if r >= 1 else 0


def _consts():
    """Host-precomputed constant tensors (input-independent).

    cstf [128, 1440] f32: ident | oidx128 | ematT | oidx9 | zc
    cstu [128, 96] u16:   idxX-T | idxS-T | idxE | pad
    """
    ident128 = np.eye(128, dtype=np.float32)
    oidx128 = np.tile((np.arange(128) % 64).astype(np.float32)[None, :],
                      (128, 1))

    # xsel gather index streams (same values as v1, host layout [128, 64]:
    # col t*8 + j//16, row 16g + j%16 for tile t, stream j).
    idxX = np.zeros((128, 64), np.uint16)
    for t in range(8):
        k2y, k2xh = t // 2, t % 2
        for g in range(8):
            k2xp = g // 4
            k1y = g % 4
            k2x = 2 * k2xh + k2xp
            for j in range(128):
                b, oy, ox = j // 64, (j % 64) // 8, j % 8
                idxX[16 * g + j % 16, t * 8 + j // 16] = (
                    b * 1444 + (4 * oy + 2 * k2y + k1y) * 38
                    + 4 * ox + 2 * k2x)

    # sel gather (2 tiles s, packed into [128, 16])
    idxS = np.zeros((128, 16), np.uint16)
    for s in range(2):
        for g in range(8):
            k2yp, k2x = g // 4, g % 4
            k2y = s * 2 + k2yp
            for j in range(128):
                b, oy, ox = j // 64, (j % 64) // 8, j % 8
                idxS[16 * g + j % 16, s * 8 + j // 16] = (
                    b * 1444 + (4 * oy + 2 * k2y) * 38 + 4 * ox + 2 * k2x)

    # expansion matrices, transposed host layout [128, 8*128]
    emat = np.zeros((8, 128, 128), np.float32)
    for t in range(8):
        k2y, k2xh = t // 2, t % 2
        for p in range(128):
            k2xp, k1y, k1x = p // 64, (p % 64) // 16, p % 4
            k2x = 2 * k2xh + k2xp
            r = (k2y % 2) * 64 + k2x * 16 + k1y * 4 + k1x
            emat[t, r, p] = 1.0
    ematT = np.transpose(emat, (1, 0, 2)).reshape(128, 1024)

    # E9 gather
    idxE = np.zeros((128, 9), np.uint16)
    for r in range(4):
        for h in range(2):
            g = r * 2 + h
            for j in range(144):
                ixl, jj = j // 9, j % 9
                jy, jx = jj // 3, jj % 3
                t_ = ixl % 4
                s = 4 * h + ixl // 4
                dy = r - 4 * _e(r) + 4 * jy + 3
                dx = t_ - 4 * _e(t_) + 4 * jx + 3
                oxp = s + _e(t_) - jx + 2
                if 0 <= dy < 10 and 0 <= dx < 10:
                    idx = (2 - jy) * 1200 + oxp * 100 + dy * 10 + dx
                else:
                    idx = 0  # guaranteed-zero pad cell
                idxE[16 * g + j % 16, j // 16] = idx

    # candidate o-index (3000 = invalid) and first-noncovering-o tables
    oidx9 = np.full((128, 144), 3000.0, np.float32)
    zc = np.zeros((128, 16), np.float32)
    for r in range(4):
        for h in range(2):
            for b in range(2):
                for q in range(8):
                    p = r * 32 + h * 16 + b * 8 + q
                    iy = 4 * q + r
                    for ixl in range(16):
                        ix = 16 * h + ixl
                        t_ = ix % 4
                        s = ix // 4
                        for jj in range(9):
                            jy, jx = jj // 3, jj % 3
                            oy = q + _e(r) - jy
                            ox = s + _e(t_) - jx
                            dy = iy - 4 * oy + 3
                            dx = ix - 4 * ox + 3
                            if (0 <= oy < 8 and 0 <= ox < 8
                                    and 0 <= dy < 10 and 0 <= dx < 10):
                                oidx9[p, ixl * 9 + jj] = oy * 8 + ox
                        for o in range(64):
                            oy, ox = o // 8, o % 8
                            if not (0 <= iy - 4 * oy + 3 < 10
                                    and 0 <= ix - 4 * ox + 3 < 10):
                                zc[p, ixl] = float(o)
                                break
    cstf = np.concatenate(
        [ident128, oidx128, oidx9, zc], axis=1)
    assert cstf.shape == (128, 416), cstf.shape
    cstu = np.concatenate(
        [idxX, idxS, idxE, np.zeros((128, 7), np.uint16)], axis=1)
    assert cstu.shape == (128, 96), cstu.shape
    import ml_dtypes
    return {"cstf": np.ascontiguousarray(cstf),
            "cstu": np.ascontiguousarray(cstu),
            "cstb": np.ascontiguousarray(ematT.astype(ml_dtypes.bfloat16))}


def build_program(nc):
    x_d = nc.declare_dram_parameter("x", [2, 3, 32, 32], F32, isOutput=False)
    w1_d = nc.declare_dram_parameter("w1", [32, 3, 4, 4], F32, isOutput=False)
    b1_d = nc.declare_dram_parameter("b1", [32], F32, isOutput=False)
    w2_d = nc.declare_dram_parameter("w2", [64, 32, 4, 4], F32, isOutput=False)
    b2_d = nc.declare_dram_parameter("b2", [64], F32, isOutput=False)
    k_d = nc.declare_dram_parameter("K", [512, 64], F32, isOutput=False)
    v_d = nc.declare_dram_parameter("V", [512, 64], F32, isOutput=False)
    cf_d = nc.declare_dram_parameter("cstf", [128, 416], F32, isOutput=False)
    cu_d = nc.declare_dram_parameter("cstu", [128, 96], U16, isOutput=False)
    cb_d = nc.declare_dram_parameter("cstb", [128, 1024], BF16,
                                     isOutput=False)
    out_d = nc.declare_dram_parameter("out", [2, 64, 8, 8], F32,
                                      isOutput=True)

    with tile.TileContext(nc) as tc:
        with (
            tc.tile_pool(name="const", bufs=1) as cpool,
            tc.tile_pool(name="work", bufs=1) as wpool,
            tc.tile_pool(name="psA", bufs=2, space="PSUM") as psA,
            tc.tile_pool(name="psB", bufs=2, space="PSUM") as psB,
            tc.tile_pool(name="psC", bufs=1, space="PSUM") as psC,
            tc.tile_pool(name="dram", bufs=1, space="DRAM") as dpool,
        ):
            dmaS = nc.sync.dma_start
            dmaA = nc.scalar.dma_start

            # ---- phase-A-critical loads first ----
            xp2 = wpool.tile([3, 2892], F32)  # [ci, (b, 38x38)] + 4 tail
            nc.vector.memset(xp2[:], 0.0)
            xpitch = xp2.ap[0][0]
            for b in range(2):
                (dmaS if b == 0 else dmaA)(
                    v(xp2, b * 1444 + 117,
                      [[xpitch, 3], [38, 32], [1, 32]]),
                    AP(x_d, b * 3072, [[1024, 3], [32, 32], [1, 32]]))

            b1t = wpool.tile([32, 1], F32)
            dmaA(b1t[:], AP(b1_d, 0, [[1, 32], [1, 1]]))
            w1sb = wpool.tile([32, 48], F32)          # [m, (ci,k1)]
            dmaA(w1sb[:], AP(w1_d, 0, [[48, 32], [1, 48]]))

            # conv2 / hopfield staging (needed a few us later)
            w2sb = wpool.tile([32, 1024], F32)        # [m, (c,k2)]
            dmaS(w2sb[:], AP(w2_d, 0, [[16, 32], [512, 64], [1, 16]]))
            w2c2 = wpool.tile([64, 512], F32)         # [c, (m,k2)]
            dmaA(w2c2[:], AP(w2_d, 0, [[512, 64], [16, 32], [1, 16]]))
            b2t = wpool.tile([64, 1], F32)
            dmaA(b2t[:], AP(b2_d, 0, [[1, 64], [1, 1]]))

            cst = cpool.tile([128, 416], F32)
            dmaS(cst[:], cf_d[:])
            ident = cst[:, 0:128]
            oidx9 = cst[:, 256:400]
            zct = cst[:, 400:416]
            emtb = cpool.tile([128, 1024], BF16)
            dmaA(emtb[:], cb_d[:])
            cstu = cpool.tile([128, 96], U16)
            dmaA(cstu[:], cu_d[:])
            ixX = cstu[:, 0:64]
            ixS = cstu[:, 64:80]
            ixE = cstu[:, 80:89]

            ktl = wpool.tile([128, 256], F32)  # K 4 col-tiles
            dmaS(ktl[:], AP(k_d, 0, [[64, 128], [8192, 4], [1, 64]]))
            v_sb = wpool.tile([128, 256], F32)
            dmaA(v_sb[:], AP(v_d, 0, [[64, 128], [8192, 4], [1, 64]]))

            # data_x base rows (needed by conv1): one overlapping-window
            # fanout DMA; rows 0:12 = (ci, k1x)-shifted padded images
            data_x = wpool.tile([128, 2888], F32)
            dxp = data_x.ap[0][0]
            nc.vector.memset(data_x[0:16, :], 0.0)
            dmaS(data_x[0:12, :],
                 v(xp2, 0, [[xpitch, 3], [1, 4], [1, 2888]]))

            # conv1 stationary w1kx [12 rows=(ci,k1x), 128 free=(m,k1y)]:
            # DVE reorder -> shift-replica fanout -> PE transpose
            w1r = wpool.tile([32, 48], F32)    # free (k1y, ci, k1x)
            nc.vector.tensor_copy(
                v(w1r, 0, [[w1r.ap[0][0], 32], [12, 4], [4, 3], [1, 4]]),
                v(w1sb, 0, [[w1sb.ap[0][0], 32], [4, 4], [16, 3], [1, 4]]))
            w1kxT = wpool.tile([128, 12], F32)  # rows (m,k1y)
            dmaA(w1kxT[:],
                 v(w1r, 0, [[w1r.ap[0][0], 32], [12, 4], [1, 12]]))
            w1kx_ps = psB.tile([12, 128], F32, tag="psB", name="w1kx_ps")
            nc.tensor.transpose(w1kx_ps[:], w1kxT[:], ident)
            w1kx = wpool.tile([12, 128], F32)
            nc.scalar.copy(w1kx[:], w1kx_ps[:])

            # w2cxT [64, 512]: free (k2y, m, k2x) reorder of w2c2
            w2cxT = wpool.tile([64, 512], F32)
            nc.vector.tensor_copy(
                v(w2cxT, 0,
                  [[w2cxT.ap[0][0], 64], [128, 4], [4, 32], [1, 4]]),
                v(w2c2, 0,
                  [[w2c2.ap[0][0], 64], [4, 4], [16, 32], [1, 4]]))
            # conv2 stationary w2x [128 rows=(m,k2x), 256]: per-half
            # fanout + transpose; half h free = (c, k2y%2)
            w2x = wpool.tile([128, 256], F32)
            w2xp = w2x.ap[0][0]
            for h2 in range(2):
                wxT = wpool.tile([128, 128], F32, name=f"wxT{h2}")
                dmaA(wxT[:],
                     v(w2cxT, h2 * 256,
                       [[w2cxT.ap[0][0], 64], [128, 2], [1, 128]]))
                wx_ps = psB.tile([128, 128], F32, tag="psB",
                                 name=f"wx_ps{h2}")
                nc.tensor.transpose(wx_ps[:], wxT[:], ident)
                nc.scalar.copy(w2x[:, h2 * 128:(h2 + 1) * 128], wx_ps[:])

            # ---- Phase A: conv1 (4 tap-merged matmuls) ----
            y1ps = psA.tile([32, 512], F32, tag="psA", name="y1ps")
            for k1y in range(4):
                nc.tensor.matmul(
                    y1ps[:],
                    v(w1kx, k1y, [[w1kx.ap[0][0], 12], [4, 32]]),
                    v(data_x, 78 + 38 * k1y,
                      [[dxp, 12], [1444, 2], [76, 16], [2, 16]]),
                    start=(k1y == 0), stop=(k1y == 3))
            y1sb = wpool.tile([32, 512], F32)  # [m, (b,py,px)]
            nc.scalar.activation(y1sb[:], y1ps[:], AF.Relu, bias=b1t[:])

            # ---- x-side staging (overlaps A/B; off critical path) ----
            # w1sb2p / w1fp build
            w1sb2 = wpool.tile([32, 48], F32)   # [m, (k1x,ci,k1y)]
            dmaA(w1sb2[:], AP(w1_d, 0, [[48, 32], [16, 3], [1, 16]]))
            w1sb2p = wpool.tile([32, 64], F32)  # [m, (k1y,ci4,k1x)]
            nc.vector.memset(w1sb2p[:], 0.0)
            nc.vector.tensor_copy(
                v(w1sb2p, 0, [[w1sb2p.ap[0][0], 32], [16, 4], [4, 3], [1, 4]]),
                v(w1sb2, 0, [[w1sb2.ap[0][0], 32], [4, 4], [16, 3], [1, 4]]))
            w1fp_ps = psB.tile([64, 32], F32, tag="psB", name="w1fp_ps")
            nc.tensor.transpose(w1fp_ps[:], w1sb2p[:], cst[0:32, 0:32])
            w1fp = wpool.tile([128, 32], F32)
            nc.scalar.copy(w1fp[0:64, :], w1fp_ps[:])
            dmaS(w1fp[64:128, :], w1fp[0:64, :])

            # data_x replicas for the x-side gather
            for d in (16, 32, 64):
                dmaA(data_x[d:2 * d, :], data_x[0:d, :])
            xgall = wpool.tile([128, 1024], F32)
            nc.gpsimd.indirect_copy(
                v(xgall, 0, [[xgall.ap[0][0], 128], [1, 1024], [1, 1]]),
                data_x[:], ixX, True)

            # K^T via 4 PE transposes
            kt_sb = wpool.tile([64, 512], F32)
            for t in range(4):
                kt_ps = psB.tile([64, 128], F32, tag="psB", name="kt_ps")
                nc.tensor.transpose(kt_ps[:], ktl[:, t * 64:(t + 1) * 64],
                                    ident)
                nc.scalar.copy(kt_sb[:, t * 128:(t + 1) * 128], kt_ps[:])

            # early staging for the argmin/sel fanouts
            z225 = cpool.tile([128, 225], F32)
            nc.vector.memset(z225[:], 0.0)
            zneg = cpool.tile([128, 95], F32)
            nc.vector.memset(zneg[:], -1.0)
            ed4p = dpool.tile([28800], F32)  # [b, 12 oyp, 12 oxp, 100]
            dmaS(v(ed4p, 0, [[225, 128], [1, 225]]), z225[:])
            sel_pad = dpool.tile([3040], F32)
            dmaA(v(sel_pad, 0, [[95, 32], [1, 95]]), zneg[0:32, :])

            # ---- Phase A: conv2 (4 tap-merged matmuls) ----
            y1p = wpool.tile([32, 652], F32)   # [m, (b,18,18)] padded
            nc.vector.memset(y1p[:], 0.0)
            ypitch = y1p.ap[0][0]
            nc.vector.tensor_copy(
                v(y1p, 19, [[ypitch, 32], [324, 2], [18, 16], [1, 16]]),
                v(y1sb, 0,
                  [[y1sb.ap[0][0], 32], [256, 2], [16, 16], [1, 16]]))
            m1p = wpool.tile([32, 652], F32)
            nc.vector.tensor_scalar(m1p[:], y1p[:], 0.0, None, ALU.is_gt)

            def tapview(tl, k2, pitch):
                k2y, k2x = k2 // 4, k2 % 4
                return v(tl, 18 * k2y + k2x,
                         [[pitch, 32], [324, 2], [36, 8], [2, 8]])

            # y1x [128 rows=(m,k2x), 648]: k2x-shift replicas of y1p
            y1x = wpool.tile([128, 648], F32)
            yxp = y1x.ap[0][0]
            dmaS(y1x[:], v(y1p, 0, [[ypitch, 32], [1, 4], [1, 648]]))

            ypre = psA.tile([64, 128], F32, tag="psA", name="ypre")
            for k2y in range(4):
                nc.tensor.matmul(
                    ypre[:],
                    v(w2x, (k2y // 2) * 128 + k2y % 2,
                      [[w2xp, 128], [2, 64]]),
                    v(y1x, 18 * k2y,
                      [[yxp, 128], [324, 2], [36, 8], [2, 8]]),
                    start=(k2y == 0), stop=(k2y == 3))
            yT = wpool.tile([64, 128], F32)    # [c, (b,o)]
            nc.scalar.activation(yT[:], ypre[:], AF.Relu, bias=b2t[:])
            m2T = wpool.tile([64, 128], F32)
            nc.vector.tensor_scalar(m2T[:], yT[:], 0.0, None, ALU.is_gt)

            # m1x [128 rows=(m,k2x), 648]: k2x-shift replicas of m1p
            m1x = wpool.tile([128, 648], F32)
            mxp = m1x.ap[0][0]
            dmaA(m1x[:], v(m1p, 0, [[m1p.ap[0][0], 32], [1, 4], [1, 648]]))

            def hopfield(src, tag):
                """src [c 64, (b,o) 128] SBUF -> yq [(b,o) 128, c 64] SBUF."""
                a_ps = psA.tile([128, 512], F32, tag="psA",
                                name=f"a_ps{tag}")
                nc.tensor.matmul(a_ps[:], src, kt_sb[:], start=True,
                                 stop=True)
                rmax = wpool.tile([128, 1], F32, name=f"rmax{tag}")
                nc.vector.tensor_reduce(rmax[:], a_ps[:], AX.X, ALU.max)
                negbm = wpool.tile([128, 1], F32, name=f"negbm{tag}")
                nc.vector.tensor_scalar(negbm[:], rmax[:], -0.125, None,
                                        ALU.mult)
                p_sb = wpool.tile([128, 512], F32, name=f"p_sb{tag}")
                ssum = wpool.tile([128, 1], F32, name=f"ssum{tag}")
                nc.scalar.activation(p_sb[:], a_ps[:], AF.Exp, bias=negbm[:],
                                     scale=0.125, accum_out=ssum[:])
                rec = wpool.tile([128, 1], F32, name=f"rec{tag}")
                nc.vector.reciprocal(rec[:], ssum[:])
                nc.vector.tensor_scalar(p_sb[:], p_sb[:], rec[:], None,
                                        ALU.mult)
                yq_ps = psC.tile([128, 64], F32, tag="psC",
                                 name=f"yq_ps{tag}")
                for t in range(4):
                    pt_ps = psB.tile([128, 128], F32, tag="psB",
                                     name=f"pt_ps{tag}{t}")
                    nc.tensor.transpose(pt_ps[:],
                                        p_sb[:, t * 128:(t + 1) * 128],
                                        ident)
                    pt_sb = wpool.tile([128, 128], F32, tag="pt_sb",
                                       name=f"pt_sb{tag}{t}")
                    nc.scalar.copy(pt_sb[:], pt_ps[:])
                    nc.tensor.matmul(yq_ps[:], pt_sb[:],
                                     v_sb[:, t * 64:(t + 1) * 64],
                                     start=(t == 0), stop=(t == 3))
                yq_sb = wpool.tile([128, 64], F32, name=f"yq_sb{tag}")
                nc.scalar.copy(yq_sb[:], yq_ps[:])
                return yq_sb

            yq1 = hopfield(yT[:], "h1")

            yqT_ps = psB.tile([64, 128], F32, tag="psB", name="yqT_ps")
            nc.tensor.transpose(yqT_ps[:], yq1[:], ident)
            r2T = wpool.tile([64, 128], F32)
            nc.vector.scalar_tensor_tensor(r2T[:], yqT_ps[:], -1.0, yT[:],
                                           ALU.mult, ALU.add)
            nc.vector.tensor_mul(r2T[:], r2T[:], m2T[:])

            # ---- Phase B: e_patch + argmin ----
            w1s16 = wpool.tile([32, 16], F32)
            nc.vector.tensor_reduce(
                w1s16[:],
                v(w1sb, 0, [[w1sb.ap[0][0], 32], [1, 16], [16, 3]]),
                AX.X, ALU.add)
            w1si = wpool.tile([32, 256], F32)  # w1s 16x16 zero-pad image
            nc.vector.memset(w1si[:], 0.0)
            nc.vector.tensor_copy(
                v(w1si, 102, [[w1si.ap[0][0], 32], [16, 4], [1, 4]]),
                w1s16[:])
            # w1si4 [128 rows=(m,k2x), 250]: -2*k2x-shifted w1si replicas
            w1si4 = wpool.tile([128, 250], F32)
            w4p = w1si4.ap[0][0]
            for k2x in range(4):
                (dmaS if k2x % 2 == 0 else dmaA)(
                    v(w1si4, k2x * w4p, [[4 * w4p, 32], [1, 250]]),
                    v(w1si, 6 - 2 * k2x, [[w1si.ap[0][0], 32], [1, 250]]))
            # w1stpX [128, 400]: per-k2y 10x10 windows
            w1stpX = wpool.tile([128, 400], F32)
            for k2y in range(4):
                nc.vector.tensor_copy(
                    v(w1stpX, k2y * 100,
                      [[w1stpX.ap[0][0], 128], [10, 10], [1, 10]]),
                    v(w1si4, (6 - 2 * k2y) * 16,
                      [[w4p, 128], [16, 10], [1, 10]]))

            # G + mask -> gm [128 rows=(m,k2x), 512 (k2y, (b,o))]
            gm = wpool.tile([128, 512], F32)
            for k2y in range(4):
                g_ps = psB.tile([128, 128], F32, tag="psB",
                                name=f"g_ps{k2y}")
                nc.tensor.matmul(
                    g_ps[:],
                    w2cxT[:, k2y * 128:(k2y + 1) * 128],
                    r2T[:], start=True, stop=True)
                nc.vector.tensor_tensor(
                    gm[:, k2y * 128:(k2y + 1) * 128], g_ps[:],
                    v(m1x, 18 * k2y,
                      [[mxp, 128], [324, 2], [36, 8], [2, 8]]),
                    ALU.mult)

            ep_ps = psA.tile([100, 128], F32, tag="psA", name="ep_ps")
            for k2y in range(4):
                nc.tensor.matmul(
                    ep_ps[:],
                    w1stpX[:, k2y * 100:(k2y + 1) * 100],
                    gm[:, k2y * 128:(k2y + 1) * 128],
                    start=(k2y == 0), stop=(k2y == 3))
            ep_sb = wpool.tile([100, 128], F32)
            nc.scalar.copy(ep_sb[:], ep_ps[:])
            ep2_ps = psB.tile([128, 100], F32, tag="psB", name="ep2_ps")
            nc.tensor.transpose(ep2_ps[:], ep_sb[:], cst[0:100, 0:100])
            ep2 = wpool.tile([128, 100], F32)  # [(b,oy,ox), (dy,dx)]
            nc.scalar.copy(ep2[:], ep2_ps[:])

            # scatter patches to DRAM (padded per-o layout)
            for b in range(2):
                (dmaS if b == 0 else dmaA)(
                    v(ed4p, b * 14400 + 2 * 1200 + 2 * 100,
                      [[1200, 8], [100, 8], [1, 100]]),
                    ep2[b * 64:(b + 1) * 64, :])

            # data_e: 8 window DMAs from DRAM
            data_e = wpool.tile([128, 3600], F32)
            for r in range(4):
                for h in range(2):
                    (dmaS if h == 0 else dmaA)(
                        data_e[(r * 2 + h) * 16:(r * 2 + h + 1) * 16, :],
                        v(ed4p, _e(r) * 1200,
                          [[14400, 2], [1200, 8], [1, 3600]]))
            e9 = wpool.tile([128, 144], F32)
            e9pitch = e9.ap[0][0]
            nc.gpsimd.indirect_copy(
                v(e9, 0, [[e9pitch, 128], [1, 144], [1, 1]]),
                data_e[:], ixE, True)

            # argmin with reference tie semantics
            mincand = wpool.tile([128, 16], F32)
            nc.vector.tensor_reduce(
                mincand[:], v(e9, 0, [[e9pitch, 128], [9, 16], [1, 9]]),
                AX.X, ALU.min)
            mstar = wpool.tile([128, 16], F32)
            nc.vector.tensor_scalar(mstar[:], mincand[:], 0.0, None, ALU.min)
            eq9 = wpool.tile([128, 144], F32)
            nc.vector.tensor_tensor(
                v(eq9, 0, [[eq9.ap[0][0], 128], [9, 16], [1, 9]]),
                v(e9, 0, [[e9pitch, 128], [9, 16], [1, 9]]),
                v(mstar, 0, [[mstar.ap[0][0], 128], [1, 16], [0, 9]]),
                ALU.is_equal)
            cs = wpool.tile([128, 144], F32)
            nc.vector.scalar_tensor_tensor(cs[:], eq9[:], -1000.0, oidx9,
                                           ALU.mult, ALU.add)
            minc2 = wpool.tile([128, 16], F32)
            nc.vector.tensor_reduce(
                minc2[:], v(cs, 0, [[cs.ap[0][0], 128], [9, 16], [1, 9]]),
                AX.X, ALU.min)
            zeq = wpool.tile([128, 16], F32)
            nc.vector.tensor_scalar(zeq[:], mstar[:], 0.0, None,
                                    ALU.is_equal)
            zsc = wpool.tile([128, 16], F32)
            nc.vector.scalar_tensor_tensor(zsc[:], zeq[:], -1000.0, zct,
                                           ALU.mult, ALU.add)
            sel16 = wpool.tile([128, 16], F32)
            nc.vector.tensor_tensor(sel16[:], minc2[:], zsc[:], ALU.min)
            nc.vector.tensor_scalar(sel16[:], sel16[:], 1000.0, None,
                                    ALU.add)

            # sel -> DRAM padded image (pad = -1, pre-filled)
            for r in range(4):
                for h in range(2):
                    (dmaS if h == 0 else dmaA)(
                        v(sel_pad, 117 + 38 * r + 16 * h,
                          [[1444, 2], [152, 8], [1, 16]]),
                        sel16[r * 32 + h * 16:r * 32 + h * 16 + 16, :])

            # ---- Phase C: sel gather + mask expansion ----
            # data_s: 8 fanout DMAs (16 shifted rows each, one per replica)
            data_s = wpool.tile([128, 2888], F32)
            for g in range(8):
                (dmaS if g % 2 == 0 else dmaA)(
                    data_s[g * 16:(g + 1) * 16, :],
                    v(sel_pad, 0, [[38, 4], [1, 4], [1, 2888]]))
            sg = wpool.tile([128, 256], F32)
            nc.gpsimd.indirect_copy(
                v(sg, 0, [[sg.ap[0][0], 128], [1, 256], [1, 1]]),
                data_s[:], ixS, True)
            sgb = wpool.tile([128, 256], BF16)
            nc.vector.tensor_tensor(
                v(sgb, 0, [[sgb.ap[0][0], 128], [128, 2], [1, 128]]),
                v(sg, 0, [[sg.ap[0][0], 128], [128, 2], [1, 128]]),
                v(cst, 128, [[cst.ap[0][0], 128], [0, 2], [1, 128]]),
                ALU.is_equal)
            selm2 = [sgb[:, 0:128], sgb[:, 128:256]]

            xsel = []
            for t in range(8):
                mx_ps = psB.tile([128, 128], F32, tag="psB", name=f"mx{t}")
                nc.tensor.matmul(mx_ps[:],
                                 emtb[:, t * 128:(t + 1) * 128],
                                 selm2[(t // 2) // 2],
                                 start=True, stop=True)
                xs = wpool.tile([128, 128], F32, name=f"xs{t}")
                nc.vector.tensor_tensor(xs[:],
                                        xgall[:, t * 128:(t + 1) * 128],
                                        mx_ps[:], ALU.mult)
                xsel.append(xs)

            zm = [wpool.tile([32, 128], F32, name=f"zm{k2}")
                  for k2 in range(16)]
            for k2 in range(16):
                k2y, k2x = k2 // 4, k2 % 4
                t = k2y * 2 + k2x // 2
                half = (k2x % 2) * 64
                z_ps = psB.tile([32, 128], F32, tag="psB", name=f"z_ps{k2}")
                nc.tensor.matmul(z_ps[:], w1fp[half:half + 64, :],
                                 xsel[t][half:half + 64, :],
                                 start=True, stop=True)
                nc.vector.tensor_tensor(zm[k2][:], z_ps[:],
                                        tapview(m1p, k2, ypitch), ALU.mult)

            ym_ps = psA.tile([128, 64], F32, tag="psA", name="ym_ps")
            for k2 in range(16):
                nc.tensor.matmul(
                    ym_ps[:], zm[k2][:],
                    v(w2sb, k2, [[w2sb.ap[0][0], 32], [16, 64]]),
                    start=(k2 == 0), stop=(k2 == 15))

            yTT_ps = psB.tile([128, 64], F32, tag="psB", name="yTT_ps")
            nc.tensor.transpose(yTT_ps[:], yT[:], cst[0:64, 0:64])
            m2g = wpool.tile([128, 64], F32)
            nc.vector.tensor_scalar(m2g[:], yTT_ps[:], 0.0, None, ALU.is_gt)
            ymm = wpool.tile([128, 64], F32)
            nc.vector.tensor_tensor(ymm[:], ym_ps[:], m2g[:], ALU.mult)

            t2_ps = psB.tile([64, 128], F32, tag="psB", name="t2_ps")
            nc.tensor.transpose(t2_ps[:], ymm[:], ident)
            ymmT = wpool.tile([64, 128], F32)
            nc.scalar.copy(ymmT[:], t2_ps[:])

            yq2 = hopfield(ymmT[:], "h2")

            tr_ps = psB.tile([64, 128], F32, tag="psB", name="tr_ps")
            nc.tensor.transpose(tr_ps[:], yq2[:], ident)
            outT = wpool.tile([64, 128], F32)
            nc.scalar.copy(outT[:], tr_ps[:])
            for b in range(2):
                (dmaS if b == 0 else dmaA)(
                    AP(out_d, b * 4096, [[64, 64], [8, 8], [1, 8]]),
                    outT[:, b * 64:(b + 1) * 64])

    return nc


_CACHE = {}


def kernel(**inputs) -> np.ndarray:
    from concourse.bass_utils import run_bass_kernel_spmd
    if "nc" not in _CACHE:
        from concourse import bacc
        nc = bacc.Bacc("TRN2", target_bir_lowering=False, debug=False,
                       num_devices=N_CORES)
        build_program(nc)
        nc.compile()
        _CACHE["nc"] = nc
        _CACHE["consts"] = _consts()
    nc = _CACHE["nc"]
    feed = {}
    for k, val in inputs.items():
        arr = np.asarray(val)
        if arr.dtype != np.uint16:
            arr = np.asarray(arr, np.float32)
        feed[k] = np.ascontiguousarray(arr)
    for k, val in _CACHE["consts"].items():
        feed[k] = val
    in_maps = [dict(feed) for _ in range(N_CORES)]
    res = run_bass_kernel_spmd(nc, in_maps, list(range(N_CORES)))
    return np.asarray(res.results[0]["out"], np.float32)


# revision 35
# speedup vs baseline: 1.9369x; 1.4030x over previous
"""Trainium2 Bass kernel for nn_Block1_87144886436577 (vq_codebook).

v2: same math as v1 (analytic jacobian collapse -> tap matmuls,
9-candidate argmin with tie rule, masked scatter via tap matmuls), but
the data plumbing is rebuilt for latency: no DRAM scratch round trips
(the per-o patch image and the sel image are staged on single SBUF
partitions and fanned out with overlapping-window SBUF->SBUF DMAs),
host constants are repacked so each loads with one contiguous DMA,
the eight x-side gathers run as one GPSIMD indirect_copy, and DMAs are
split across the two HWDGE queues (sync + scalar).

Single-core program; all 8 cores run identical replicas. Output read
from core 0.
"""
import sys

import numpy as np

for _p in ("/opt/trn_rl_repo",):
    if _p not in sys.path:
        sys.path.insert(0, _p)

import concourse.bass as bass
import concourse.mybir as mybir
import concourse.tile as tile

F32 = mybir.dt.float32
BF16 = mybir.dt.bfloat16
U16 = mybir.dt.uint16
AF = mybir.ActivationFunctionType
ALU = mybir.AluOpType
AX = mybir.AxisListType
AP = bass.AP

N_CORES = 8


def v(t, off, pat):
    """Custom-view AP over a tile (t = AP returned by pool.tile)."""
    return AP(t.tensor, t.offset + off, pat)


def _e(r):
    return 1 if r >= 1 else 0


def _consts():
    """Host-precomputed constant tensors (input-independent).

    cstf [128, 1440] f32: ident | oidx128 | ematT | oidx9 | zc
    cstu [128, 96] u16:   idxX-T | idxS-T | idxE | pad
    """
    ident128 = np.eye(128, dtype=np.float32)
    oidx128 = np.tile((np.arange(128) % 64).astype(np.float32)[None, :],
                      (128, 1))

    # xsel gather index streams (same values as v1, host layout [128, 64]:
    # col t*8 + j//16, row 16g + j%16 for tile t, stream j).
    idxX = np.zeros((128, 64), np.uint16)
    for t in range(8):
        k2y, k2xh = t // 2, t % 2
        for g in range(8):
            k2xp = g // 4
            k1y = g % 4
            k2x = 2 * k2xh + k2xp
            for j in range(128):
                b, oy, ox = j // 64, (j % 64) // 8, j % 8
                idxX[16 * g + j % 16, t * 8 + j // 16] = (
                    b * 1444 + (4 * oy + 2 * k2y + k1y) * 38
                    + 4 * ox + 2 * k2x)

    # candidate-min validity offsets: 25 lex-ordered (a, c) shifts
    BIG = 1.0e30
    cands = [(a, c) for a in range(-2, 3) for c in range(-2, 3)]
    CAND = np.full((128, 2500), BIG, np.float32)
    for p in range(128):
        b_, oy, ox = p // 64, (p % 64) // 8, p % 8
        for ci_, (a, c) in enumerate(cands):
            for dy in range(10):
                for dx in range(10):
                    iy, ix = 4 * oy + dy - 3, 4 * ox + dx - 3
                    ok = (0 <= iy < 32 and 0 <= ix < 32
                          and 0 <= oy + a < 8 and 0 <= ox + c < 8
                          and 0 <= dy - 4 * a < 10 and 0 <= dx - 4 * c < 10)
                    if ok:
                        CAND[p, ci_ * 100 + dy * 10 + dx] = 0.0

    # mask expansion matrices: PT[dydx, t*128 + p]
    PT = np.zeros((128, 1024), np.float32)
    for t in range(8):
        k2y, k2xh = t // 2, t % 2
        for p in range(128):
            k2xp, k1y, k1x = p // 64, (p % 64) // 16, p % 4
            k2x = 2 * k2xh + k2xp
            dy, dx = 2 * k2y + k1y, 2 * k2x + k1x
            PT[dy * 10 + dx, t * 128 + p] = 1.0

    identZ = np.zeros((128, 384), np.float32)
    identZ[:, 128:256] = np.eye(128, dtype=np.float32)
    cstf = np.concatenate([identZ, CAND], axis=1)
    assert cstf.shape == (128, 2884), cstf.shape
    import ml_dtypes
    return {"cstf": np.ascontiguousarray(cstf),
            "cstu": np.ascontiguousarray(idxX),
            "cstb": np.ascontiguousarray(PT.astype(ml_dtypes.bfloat16))}


def build_program(nc):
    x_d = nc.declare_dram_parameter("x", [2, 3, 32, 32], F32, isOutput=False)
    w1_d = nc.declare_dram_parameter("w1", [32, 3, 4, 4], F32, isOutput=False)
    b1_d = nc.declare_dram_parameter("b1", [32], F32, isOutput=False)
    w2_d = nc.declare_dram_parameter("w2", [64, 32, 4, 4], F32, isOutput=False)
    b2_d = nc.declare_dram_parameter("b2", [64], F32, isOutput=False)
    k_d = nc.declare_dram_parameter("K", [512, 64], F32, isOutput=False)
    v_d = nc.declare_dram_parameter("V", [512, 64], F32, isOutput=False)
    cf_d = nc.declare_dram_parameter("cstf", [128, 2884], F32,
                                     isOutput=False)
    cu_d = nc.declare_dram_parameter("cstu", [128, 64], U16,
                                     isOutput=False)
    cb_d = nc.declare_dram_parameter("cstb", [128, 1024], BF16,
                                     isOutput=False)
    out_d = nc.declare_dram_parameter("out", [2, 64, 8, 8], F32,
                                      isOutput=True)

    with tile.TileContext(nc) as tc:
        with (
            tc.tile_pool(name="const", bufs=1) as cpool,
            tc.tile_pool(name="work", bufs=1) as wpool,
            tc.tile_pool(name="psA", bufs=2, space="PSUM") as psA,
            tc.tile_pool(name="psB", bufs=2, space="PSUM") as psB,
            tc.tile_pool(name="psC", bufs=1, space="PSUM") as psC,
        ):
            dmaS = nc.sync.dma_start
            dmaA = nc.scalar.dma_start

            # ---- phase-A-critical loads first ----
            xp2 = wpool.tile([3, 2892], F32)  # [ci, (b, 38x38)] + 4 tail
            nc.vector.memset(xp2[:], 0.0)
            xpitch = xp2.ap[0][0]
            for b in range(2):
                (dmaS if b == 0 else dmaA)(
                    v(xp2, b * 1444 + 117,
                      [[xpitch, 3], [38, 32], [1, 32]]),
                    AP(x_d, b * 3072, [[1024, 3], [32, 32], [1, 32]]))

            b1t = wpool.tile([32, 1], F32)
            dmaA(b1t[:], AP(b1_d, 0, [[1, 32], [1, 1]]))
            w1sb = wpool.tile([32, 48], F32)          # [m, (ci,k1)]
            dmaA(w1sb[:], AP(w1_d, 0, [[48, 32], [1, 48]]))

            # conv2 / hopfield staging (needed a few us later)
            w2sb = wpool.tile([32, 1024], F32)        # [m, (c,k2)]
            dmaS(w2sb[:], AP(w2_d, 0, [[16, 32], [512, 64], [1, 16]]))
            w2c2 = wpool.tile([64, 512], F32)         # [c, (m,k2)]
            dmaA(w2c2[:], AP(w2_d, 0, [[512, 64], [16, 32], [1, 16]]))
            b2t = wpool.tile([64, 1], F32)
            dmaA(b2t[:], AP(b2_d, 0, [[1, 64], [1, 1]]))

            cst = cpool.tile([128, 2884], F32)
            dmaS(cst[:, 0:384], AP(cf_d, 0, [[2884, 128], [1, 384]]))
            dmaA(cst[:, 384:2884], AP(cf_d, 384, [[2884, 128], [1, 2500]]))
            ident = cst[:, 128:256]
            ptb = cpool.tile([128, 1024], BF16)
            dmaA(ptb[:], cb_d[:])
            cstu = cpool.tile([128, 64], U16)
            dmaA(cstu[:], cu_d[:])
            ixX = cstu[:, 0:64]

            ktl = wpool.tile([128, 256], F32)  # K 4 col-tiles
            dmaS(ktl[:], AP(k_d, 0, [[64, 128], [8192, 4], [1, 64]]))
            v_sb = wpool.tile([128, 256], F32)
            dmaA(v_sb[:], AP(v_d, 0, [[64, 128], [8192, 4], [1, 64]]))

            # data_x base rows (needed by conv1): one overlapping-window
            # fanout DMA; rows 0:12 = (ci, k1x)-shifted padded images
            data_x = wpool.tile([128, 2888], F32)
            dxp = data_x.ap[0][0]
            nc.vector.memset(data_x[0:16, :], 0.0)
            dmaS(data_x[0:12, :],
                 v(xp2, 0, [[xpitch, 3], [1, 4], [1, 2888]]))

            # conv1 stationary w1kx [12 rows=(ci,k1x), 128 free=(m,k1y)]:
            # DVE reorder -> shift-replica fanout -> PE transpose
            w1r = wpool.tile([32, 48], F32)    # free (k1y, ci, k1x)
            nc.vector.tensor_copy(
                v(w1r, 0, [[w1r.ap[0][0], 32], [12, 4], [4, 3], [1, 4]]),
                v(w1sb, 0, [[w1sb.ap[0][0], 32], [4, 4], [16, 3], [1, 4]]))
            w1kxT = wpool.tile([128, 12], F32)  # rows (m,k1y)
            dmaA(w1kxT[:],
                 v(w1r, 0, [[w1r.ap[0][0], 32], [12, 4], [1, 12]]))
            w1kx_ps = psB.tile([12, 128], F32, tag="psB", name="w1kx_ps")
            nc.tensor.transpose(w1kx_ps[:], w1kxT[:], ident)
            w1kx = wpool.tile([12, 128], F32)
            nc.scalar.copy(w1kx[:], w1kx_ps[:])

            # w2cxT [64, 512]: free (k2y, m, k2x) reorder of w2c2
            w2cxT = wpool.tile([64, 512], F32)
            nc.vector.tensor_copy(
                v(w2cxT, 0,
                  [[w2cxT.ap[0][0], 64], [128, 4], [4, 32], [1, 4]]),
                v(w2c2, 0,
                  [[w2c2.ap[0][0], 64], [4, 4], [16, 32], [1, 4]]))
            # conv2 stationary w2x [128 rows=(m,k2x), 256]: per-half
            # fanout + transpose; half h free = (c, k2y%2)
            w2x = wpool.tile([128, 256], F32)
            w2xp = w2x.ap[0][0]
            for h2 in range(2):
                wxT = wpool.tile([128, 128], F32, name=f"wxT{h2}")
                dmaA(wxT[:],
                     v(w2cxT, h2 * 256,
                       [[w2cxT.ap[0][0], 64], [128, 2], [1, 128]]))
                wx_ps = psB.tile([128, 128], F32, tag="psB",
                                 name=f"wx_ps{h2}")
                nc.tensor.transpose(wx_ps[:], wxT[:], ident)
                nc.scalar.copy(w2x[:, h2 * 128:(h2 + 1) * 128], wx_ps[:])

            # ---- Phase A: conv1 (4 tap-merged matmuls) ----
            y1ps = psA.tile([32, 512], F32, tag="psA", name="y1ps")
            for k1y in range(4):
                nc.tensor.matmul(
                    y1ps[:],
                    v(w1kx, k1y, [[w1kx.ap[0][0], 12], [4, 32]]),
                    v(data_x, 78 + 38 * k1y,
                      [[dxp, 12], [1444, 2], [76, 16], [2, 16]]),
                    start=(k1y == 0), stop=(k1y == 3))
            y1sb = wpool.tile([32, 512], F32)  # [m, (b,py,px)]
            nc.scalar.activation(y1sb[:], y1ps[:], AF.Relu, bias=b1t[:])

            # ---- x-side staging (overlaps A/B; off critical path) ----
            # w1sb2p / w1fp build
            w1sb2 = wpool.tile([32, 48], F32)   # [m, (k1x,ci,k1y)]
            dmaA(w1sb2[:], AP(w1_d, 0, [[48, 32], [16, 3], [1, 16]]))
            w1sb2p = wpool.tile([32, 64], F32)  # [m, (k1y,ci4,k1x)]
            nc.vector.memset(w1sb2p[:], 0.0)
            nc.vector.tensor_copy(
                v(w1sb2p, 0, [[w1sb2p.ap[0][0], 32], [16, 4], [4, 3], [1, 4]]),
                v(w1sb2, 0, [[w1sb2.ap[0][0], 32], [4, 4], [16, 3], [1, 4]]))
            w1fp_ps = psB.tile([64, 32], F32, tag="psB", name="w1fp_ps")
            nc.tensor.transpose(w1fp_ps[:], w1sb2p[:], cst[0:32, 128:160])
            w1fp = wpool.tile([128, 32], F32)
            nc.scalar.copy(w1fp[0:64, :], w1fp_ps[:])
            dmaS(w1fp[64:128, :], w1fp[0:64, :])

            # data_x replicas for the x-side gather
            for d in (16, 32, 64):
                dmaA(data_x[d:2 * d, :], data_x[0:d, :])
            xgall = wpool.tile([128, 1024], F32)
            nc.gpsimd.indirect_copy(
                v(xgall, 0, [[xgall.ap[0][0], 128], [1, 1024], [1, 1]]),
                data_x[:], ixX, True)

            # K^T via 4 PE transposes
            kt_sb = wpool.tile([64, 512], F32)
            for t in range(4):
                kt_ps = psB.tile([64, 128], F32, tag="psB", name="kt_ps")
                nc.tensor.transpose(kt_ps[:], ktl[:, t * 64:(t + 1) * 64],
                                    ident)
                nc.scalar.copy(kt_sb[:, t * 128:(t + 1) * 128], kt_ps[:])

            # ---- Phase A: conv2 (4 tap-merged matmuls) ----
            y1p = wpool.tile([32, 652], F32)   # [m, (b,18,18)] padded
            nc.vector.memset(y1p[:], 0.0)
            ypitch = y1p.ap[0][0]
            nc.vector.tensor_copy(
                v(y1p, 19, [[ypitch, 32], [324, 2], [18, 16], [1, 16]]),
                v(y1sb, 0,
                  [[y1sb.ap[0][0], 32], [256, 2], [16, 16], [1, 16]]))
            m1p = wpool.tile([32, 652], F32)
            nc.vector.tensor_scalar(m1p[:], y1p[:], 0.0, None, ALU.is_gt)

            def tapview(tl, k2, pitch):
                k2y, k2x = k2 // 4, k2 % 4
                return v(tl, 18 * k2y + k2x,
                         [[pitch, 32], [324, 2], [36, 8], [2, 8]])

            # y1x [128 rows=(m,k2x), 648]: k2x-shift replicas of y1p
            y1x = wpool.tile([128, 648], F32)
            yxp = y1x.ap[0][0]
            dmaS(y1x[:], v(y1p, 0, [[ypitch, 32], [1, 4], [1, 648]]))

            ypre = psA.tile([64, 128], F32, tag="psA", name="ypre")
            for k2y in range(4):
                nc.tensor.matmul(
                    ypre[:],
                    v(w2x, (k2y // 2) * 128 + k2y % 2,
                      [[w2xp, 128], [2, 64]]),
                    v(y1x, 18 * k2y,
                      [[yxp, 128], [324, 2], [36, 8], [2, 8]]),
                    start=(k2y == 0), stop=(k2y == 3))
            yT = wpool.tile([64, 128], F32)    # [c, (b,o)]
            nc.scalar.activation(yT[:], ypre[:], AF.Relu, bias=b2t[:])
            m2T = wpool.tile([64, 128], F32)
            nc.vector.tensor_scalar(m2T[:], yT[:], 0.0, None, ALU.is_gt)

            # m1x [128 rows=(m,k2x), 648]: k2x-shift replicas of m1p
            m1x = wpool.tile([128, 648], F32)
            mxp = m1x.ap[0][0]
            dmaA(m1x[:], v(m1p, 0, [[m1p.ap[0][0], 32], [1, 4], [1, 648]]))

            def hopfield(src, tag):
                """src [c 64, (b,o) 128] SBUF -> yq [(b,o) 128, c 64] SBUF."""
                a_ps = psA.tile([128, 512], F32, tag="psA",
                                name=f"a_ps{tag}")
                nc.tensor.matmul(a_ps[:], src, kt_sb[:], start=True,
                                 stop=True)
                rmax = wpool.tile([128, 1], F32, name=f"rmax{tag}")
                nc.vector.tensor_reduce(rmax[:], a_ps[:], AX.X, ALU.max)
                negbm = wpool.tile([128, 1], F32, name=f"negbm{tag}")
                nc.vector.tensor_scalar(negbm[:], rmax[:], -0.125, None,
                                        ALU.mult)
                p_sb = wpool.tile([128, 512], F32, name=f"p_sb{tag}")
                ssum = wpool.tile([128, 1], F32, name=f"ssum{tag}")
                nc.scalar.activation(p_sb[:], a_ps[:], AF.Exp, bias=negbm[:],
                                     scale=0.125, accum_out=ssum[:])
                rec = wpool.tile([128, 1], F32, name=f"rec{tag}")
                nc.vector.reciprocal(rec[:], ssum[:])
                nc.vector.tensor_scalar(p_sb[:], p_sb[:], rec[:], None,
                                        ALU.mult)
                yq_ps = psC.tile([128, 64], F32, tag="psC",
                                 name=f"yq_ps{tag}")
                for t in range(4):
                    pt_ps = psB.tile([128, 128], F32, tag="psB",
                                     name=f"pt_ps{tag}{t}")
                    nc.tensor.transpose(pt_ps[:],
                                        p_sb[:, t * 128:(t + 1) * 128],
                                        ident)
                    pt_sb = wpool.tile([128, 128], F32, tag="pt_sb",
                                       name=f"pt_sb{tag}{t}")
                    nc.scalar.copy(pt_sb[:], pt_ps[:])
                    nc.tensor.matmul(yq_ps[:], pt_sb[:],
                                     v_sb[:, t * 64:(t + 1) * 64],
                                     start=(t == 0), stop=(t == 3))
                yq_sb = wpool.tile([128, 64], F32, name=f"yq_sb{tag}")
                nc.scalar.copy(yq_sb[:], yq_ps[:])
                return yq_sb

            yq1 = hopfield(yT[:], "h1")

            yqT_ps = psB.tile([64, 128], F32, tag="psB", name="yqT_ps")
            nc.tensor.transpose(yqT_ps[:], yq1[:], ident)
            r2T = wpool.tile([64, 128], F32)
            nc.vector.scalar_tensor_tensor(r2T[:], yqT_ps[:], -1.0, yT[:],
                                           ALU.mult, ALU.add)
            nc.vector.tensor_mul(r2T[:], r2T[:], m2T[:])

            # ---- Phase B: e_patch + argmin ----
            w1s16 = wpool.tile([32, 16], F32)
            nc.vector.tensor_reduce(
                w1s16[:],
                v(w1sb, 0, [[w1sb.ap[0][0], 32], [1, 16], [16, 3]]),
                AX.X, ALU.add)
            w1si = wpool.tile([32, 256], F32)  # w1s 16x16 zero-pad image
            nc.vector.memset(w1si[:], 0.0)
            nc.vector.tensor_copy(
                v(w1si, 102, [[w1si.ap[0][0], 32], [16, 4], [1, 4]]),
                w1s16[:])
            # w1si4 [128 rows=(m,k2x), 250]: -2*k2x-shifted w1si replicas
            w1si4 = wpool.tile([128, 250], F32)
            w4p = w1si4.ap[0][0]
            for k2x in range(4):
                (dmaS if k2x % 2 == 0 else dmaA)(
                    v(w1si4, k2x * w4p, [[4 * w4p, 32], [1, 250]]),
                    v(w1si, 6 - 2 * k2x, [[w1si.ap[0][0], 32], [1, 250]]))
            # w1stpX [128, 400]: per-k2y 10x10 windows
            w1stpX = wpool.tile([128, 400], F32)
            for k2y in range(4):
                nc.vector.tensor_copy(
                    v(w1stpX, k2y * 100,
                      [[w1stpX.ap[0][0], 128], [10, 10], [1, 10]]),
                    v(w1si4, (6 - 2 * k2y) * 16,
                      [[w4p, 128], [16, 10], [1, 10]]))

            # G + mask -> gm [128 rows=(m,k2x), 512 (k2y, (b,o))]
            gm = wpool.tile([128, 512], F32)
            for k2y in range(4):
                g_ps = psB.tile([128, 128], F32, tag="psB",
                                name=f"g_ps{k2y}")
                nc.tensor.matmul(
                    g_ps[:],
                    w2cxT[:, k2y * 128:(k2y + 1) * 128],
                    r2T[:], start=True, stop=True)
                nc.vector.tensor_tensor(
                    gm[:, k2y * 128:(k2y + 1) * 128], g_ps[:],
                    v(m1x, 18 * k2y,
                      [[mxp, 128], [324, 2], [36, 8], [2, 8]]),
                    ALU.mult)

            ep_ps = psA.tile([100, 128], F32, tag="psA", name="ep_ps")
            for k2y in range(4):
                nc.tensor.matmul(
                    ep_ps[:],
                    w1stpX[:, k2y * 100:(k2y + 1) * 100],
                    gm[:, k2y * 128:(k2y + 1) * 128],
                    start=(k2y == 0), stop=(k2y == 3))
            ep_sb = wpool.tile([100, 128], F32)
            nc.scalar.copy(ep_sb[:], ep_ps[:])
            ep2_ps = psB.tile([128, 100], F32, tag="psB", name="ep2_ps")
            nc.tensor.transpose(ep2_ps[:], ep_sb[:], cst[0:100, 128:228])
            # zero-padded ep2 for shifted candidate reads
            ep2p = wpool.tile([128, 300], F32)
            nc.vector.memset(ep2p[:], 0.0)
            nc.scalar.copy(ep2p[:, 100:200], ep2_ps[:])
            e2p = ep2p.ap[0][0]

            # 25-candidate stack: stk[:, j*100:(j+1)*100] = shifted value
            # + validity offset (BIG where invalid); lex order (a, c)
            cands = [(a, c) for a in range(-2, 3) for c in range(-2, 3)]
            stk = wpool.tile([128, 2500], F32)
            cp = cst.ap[0][0]
            for j, (a, c) in enumerate(cands):
                cnd = cst[:, 384 + j * 100:384 + (j + 1) * 100]
                if (a, c) == (0, 0):
                    nc.vector.tensor_tensor(
                        stk[:, j * 100:(j + 1) * 100],
                        ep2p[:, 100:200], cnd, ALU.add)
                    continue
                dpp = 8 * a + c
                dff = -40 * a - 4 * c
                t_ps = psB.tile([128, 100], F32, tag="psB",
                                name=f"t_ps{j}")
                nc.tensor.matmul(
                    t_ps[:],
                    v(cst, 128 + dpp, [[cp, 128], [1, 128]]),
                    v(ep2p, 100 + dff, [[e2p, 128], [1, 100]]),
                    start=True, stop=True)
                nc.vector.tensor_tensor(
                    stk[:, j * 100:(j + 1) * 100], t_ps[:], cnd, ALU.add)

            vmin = wpool.tile([128, 100], F32)
            skp = stk.ap[0][0]
            nc.vector.tensor_reduce(
                vmin[:], v(stk, 0, [[skp, 128], [1, 100], [100, 25]]),
                AX.X, ALU.min)
            emin = wpool.tile([128, 100], F32)
            nc.vector.tensor_reduce(
                emin[:], v(stk, 0, [[skp, 128], [1, 100], [100, 12]]),
                AX.X, ALU.min)
            # msk = (self == vmin) & (vmin < 0) & (emin > vmin)
            weq = wpool.tile([128, 100], F32)
            nc.vector.tensor_tensor(weq[:], stk[:, 1200:1300], vmin[:],
                                    ALU.is_equal)
            wgt = wpool.tile([128, 100], F32)
            nc.vector.tensor_tensor(wgt[:], emin[:], vmin[:], ALU.is_gt)
            wneg = wpool.tile([128, 100], F32)
            nc.vector.tensor_scalar(wneg[:], vmin[:], 0.0, None, ALU.is_lt)
            msk = wpool.tile([128, 100], F32)
            nc.vector.tensor_tensor(msk[:], weq[:], wneg[:], ALU.mult)
            nc.vector.tensor_tensor(msk[:], msk[:], wgt[:], ALU.mult)

            # transpose to patch-major + bf16 for the expansion matmuls
            mskT_ps = psB.tile([100, 128], F32, tag="psB", name="mskT_ps")
            nc.tensor.transpose(mskT_ps[:], msk[:], ident)
            mskTb = wpool.tile([100, 128], BF16)
            nc.scalar.copy(mskTb[:], mskT_ps[:])

            # ---- Phase C: mask expansion + masked conv ----
            xsel = []
            for t in range(8):
                mx_ps = psB.tile([128, 128], F32, tag="psB", name=f"mx{t}")
                nc.tensor.matmul(mx_ps[:],
                                 ptb[0:100, t * 128:(t + 1) * 128],
                                 mskTb[:], start=True, stop=True)
                xs = wpool.tile([128, 128], F32, name=f"xs{t}")
                nc.vector.tensor_tensor(xs[:],
                                        xgall[:, t * 128:(t + 1) * 128],
                                        mx_ps[:], ALU.mult)
                xsel.append(xs)

            zm = [wpool.tile([32, 128], F32, name=f"zm{k2}")
                  for k2 in range(16)]
            for k2 in range(16):
                k2y, k2x = k2 // 4, k2 % 4
                t = k2y * 2 + k2x // 2
                half = (k2x % 2) * 64
                z_ps = psB.tile([32, 128], F32, tag="psB", name=f"z_ps{k2}")
                nc.tensor.matmul(z_ps[:], w1fp[half:half + 64, :],
                                 xsel[t][half:half + 64, :],
                                 start=True, stop=True)
                nc.vector.tensor_tensor(zm[k2][:], z_ps[:],
                                        tapview(m1p, k2, ypitch), ALU.mult)

            ym_ps = psA.tile([128, 64], F32, tag="psA", name="ym_ps")
            for k2 in range(16):
                nc.tensor.matmul(
                    ym_ps[:], zm[k2][:],
                    v(w2sb, k2, [[w2sb.ap[0][0], 32], [16, 64]]),
                    start=(k2 == 0), stop=(k2 == 15))

            yTT_ps = psB.tile([128, 64], F32, tag="psB", name="yTT_ps")
            nc.tensor.transpose(yTT_ps[:], yT[:], cst[0:64, 128:192])
            m2g = wpool.tile([128, 64], F32)
            nc.vector.tensor_scalar(m2g[:], yTT_ps[:], 0.0, None, ALU.is_gt)
            ymm = wpool.tile([128, 64], F32)
            nc.vector.tensor_tensor(ymm[:], ym_ps[:], m2g[:], ALU.mult)

            t2_ps = psB.tile([64, 128], F32, tag="psB", name="t2_ps")
            nc.tensor.transpose(t2_ps[:], ymm[:], ident)
            ymmT = wpool.tile([64, 128], F32)
            nc.scalar.copy(ymmT[:], t2_ps[:])

            yq2 = hopfield(ymmT[:], "h2")

            tr_ps = psB.tile([64, 128], F32, tag="psB", name="tr_ps")
            nc.tensor.transpose(tr_ps[:], yq2[:], ident)
            outT = wpool.tile([64, 128], F32)
            nc.scalar.copy(outT[:], tr_ps[:])
            for b in range(2):
                (dmaS if b == 0 else dmaA)(
                    AP(out_d, b * 4096, [[64, 64], [8, 8], [1, 8]]),
                    outT[:, b * 64:(b + 1) * 64])

    return nc


_CACHE = {}


def kernel(**inputs) -> np.ndarray:
    from concourse.bass_utils import run_bass_kernel_spmd
    if "nc" not in _CACHE:
        from concourse import bacc
        nc = bacc.Bacc("TRN2", target_bir_lowering=False, debug=False,
                       num_devices=N_CORES)
        build_program(nc)
        nc.compile()
        _CACHE["nc"] = nc
        _CACHE["consts"] = _consts()
    nc = _CACHE["nc"]
    feed = {}
    for k, val in inputs.items():
        arr = np.asarray(val)
        if arr.dtype != np.uint16:
            arr = np.asarray(arr, np.float32)
        feed[k] = np.ascontiguousarray(arr)
    for k, val in _CACHE["consts"].items():
        feed[k] = val
    in_maps = [dict(feed) for _ in range(N_CORES)]
    res = run_bass_kernel_spmd(nc, in_maps, list(range(N_CORES)))
    return np.asarray(res.results[0]["out"], np.float32)


# revision 40
# speedup vs baseline: 2.3985x; 1.2384x over previous
"""Trainium2 Bass kernel for nn_Block1_87144886436577 (vq_codebook).

v2: same math as v1 (analytic jacobian collapse -> tap matmuls,
9-candidate argmin with tie rule, masked scatter via tap matmuls), but
the data plumbing is rebuilt for latency: no DRAM scratch round trips
(the per-o patch image and the sel image are staged on single SBUF
partitions and fanned out with overlapping-window SBUF->SBUF DMAs),
host constants are repacked so each loads with one contiguous DMA,
the eight x-side gathers run as one GPSIMD indirect_copy, and DMAs are
split across the two HWDGE queues (sync + scalar).

Single-core program; all 8 cores run identical replicas. Output read
from core 0.
"""
import sys

import numpy as np

for _p in ("/opt/trn_rl_repo",):
    if _p not in sys.path:
        sys.path.insert(0, _p)

import concourse.bass as bass
import concourse.mybir as mybir
import concourse.tile as tile

F32 = mybir.dt.float32
BF16 = mybir.dt.bfloat16
U16 = mybir.dt.uint16
AF = mybir.ActivationFunctionType
ALU = mybir.AluOpType
AX = mybir.AxisListType
AP = bass.AP

N_CORES = 8


def v(t, off, pat):
    """Custom-view AP over a tile (t = AP returned by pool.tile)."""
    return AP(t.tensor, t.offset + off, pat)


def _e(r):
    return 1 if r >= 1 else 0


def _consts():
    """Host-precomputed constant tensors (input-independent).

    cstf [128, 1440] f32: ident | oidx128 | ematT | oidx9 | zc
    cstu [128, 96] u16:   idxX-T | idxS-T | idxE | pad
    """
    ident128 = np.eye(128, dtype=np.float32)
    oidx128 = np.tile((np.arange(128) % 64).astype(np.float32)[None, :],
                      (128, 1))

    # xsel gather index streams (same values as v1, host layout [128, 64]:
    # col t*8 + j//16, row 16g + j%16 for tile t, stream j).
    idxX = np.zeros((128, 64), np.uint16)
    for t in range(8):
        k2y, k2xh = t // 2, t % 2
        for g in range(8):
            k2xp = g // 4
            k1y = g % 4
            k2x = 2 * k2xh + k2xp
            for j in range(128):
                b, oy, ox = j // 64, (j % 64) // 8, j % 8
                idxX[16 * g + j % 16, t * 8 + j // 16] = (
                    b * 1444 + (4 * oy + 2 * k2y + k1y) * 38
                    + 4 * ox + 2 * k2x)

    # candidate-min validity offsets: 25 lex-ordered (a, c) shifts
    BIG = 1.0e30
    cands = [(a, c) for a in range(-2, 3) for c in range(-2, 3)]
    CAND = np.full((128, 2500), BIG, np.float32)
    for p in range(128):
        b_, oy, ox = p // 64, (p % 64) // 8, p % 8
        for ci_, (a, c) in enumerate(cands):
            for dy in range(10):
                for dx in range(10):
                    iy, ix = 4 * oy + dy - 3, 4 * ox + dx - 3
                    ok = (0 <= iy < 32 and 0 <= ix < 32
                          and 0 <= oy + a < 8 and 0 <= ox + c < 8
                          and 0 <= dy - 4 * a < 10 and 0 <= dx - 4 * c < 10)
                    if ok:
                        CAND[p, ci_ * 100 + dy * 10 + dx] = 0.0

    # mask expansion matrices: PT[dydx, t*128 + p]
    PT = np.zeros((128, 1024), np.float32)
    for t in range(8):
        k2y, k2xh = t // 2, t % 2
        for p in range(128):
            k2xp, k1y, k1x = p // 64, (p % 64) // 16, p % 4
            k2x = 2 * k2xh + k2xp
            dy, dx = 2 * k2y + k1y, 2 * k2x + k1x
            PT[dy * 10 + dx, t * 128 + p] = 1.0

    identZ = np.zeros((128, 384), np.float32)
    identZ[:, 128:256] = np.eye(128, dtype=np.float32)
    cstf = np.concatenate([identZ, CAND], axis=1)
    assert cstf.shape == (128, 2884), cstf.shape
    import ml_dtypes
    cstb = np.concatenate(
        [PT, np.eye(128, dtype=np.float32)], axis=1)
    return {"cstf": np.ascontiguousarray(cstf),
            "cstu": np.ascontiguousarray(idxX),
            "cstb": np.ascontiguousarray(cstb.astype(ml_dtypes.bfloat16))}


def build_program(nc):
    x_d = nc.declare_dram_parameter("x", [2, 3, 32, 32], F32, isOutput=False)
    w1_d = nc.declare_dram_parameter("w1", [32, 3, 4, 4], F32, isOutput=False)
    b1_d = nc.declare_dram_parameter("b1", [32], F32, isOutput=False)
    w2_d = nc.declare_dram_parameter("w2", [64, 32, 4, 4], F32, isOutput=False)
    b2_d = nc.declare_dram_parameter("b2", [64], F32, isOutput=False)
    k_d = nc.declare_dram_parameter("K", [512, 64], F32, isOutput=False)
    v_d = nc.declare_dram_parameter("V", [512, 64], F32, isOutput=False)
    cf_d = nc.declare_dram_parameter("cstf", [128, 2884], F32,
                                     isOutput=False)
    cu_d = nc.declare_dram_parameter("cstu", [128, 64], U16,
                                     isOutput=False)
    cb_d = nc.declare_dram_parameter("cstb", [128, 1152], BF16,
                                     isOutput=False)
    out_d = nc.declare_dram_parameter("out", [2, 64, 8, 8], F32,
                                      isOutput=True)

    with tile.TileContext(nc) as tc:
        with (
            tc.tile_pool(name="const", bufs=1) as cpool,
            tc.tile_pool(name="work", bufs=1) as wpool,
            tc.tile_pool(name="psA", bufs=2, space="PSUM") as psA,
            tc.tile_pool(name="psB", bufs=2, space="PSUM") as psB,
            tc.tile_pool(name="psC", bufs=1, space="PSUM") as psC,
            tc.tile_pool(name="psD", bufs=1, space="PSUM") as psD,
        ):
            dmaS = nc.sync.dma_start
            dmaA = nc.scalar.dma_start

            # ---- phase-A-critical loads first ----
            xp2 = wpool.tile([3, 2892], F32)  # [ci, (b, 38x38)] + 4 tail
            nc.vector.memset(xp2[:], 0.0)
            xpitch = xp2.ap[0][0]
            for b in range(2):
                (dmaS if b == 0 else dmaA)(
                    v(xp2, b * 1444 + 117,
                      [[xpitch, 3], [38, 32], [1, 32]]),
                    AP(x_d, b * 3072, [[1024, 3], [32, 32], [1, 32]]))

            b1t = wpool.tile([32, 1], F32)
            dmaA(b1t[:], AP(b1_d, 0, [[1, 32], [1, 1]]))
            w1sb = wpool.tile([32, 48], F32)          # [m, (ci,k1)]
            dmaA(w1sb[:], AP(w1_d, 0, [[48, 32], [1, 48]]))

            # conv2 / hopfield staging (needed a few us later)
            w2sb = wpool.tile([32, 1024], BF16)       # [m, (c,k2)]
            nc.gpsimd.dma_start(
                w2sb[:], AP(w2_d, 0, [[16, 32], [512, 64], [1, 16]]))
            w2c2 = wpool.tile([64, 512], F32)         # [c, (m,k2)]
            dmaA(w2c2[:], AP(w2_d, 0, [[512, 64], [16, 32], [1, 16]]))
            b2t = wpool.tile([64, 1], F32)
            dmaA(b2t[:], AP(b2_d, 0, [[1, 64], [1, 1]]))

            cst = cpool.tile([128, 2884], F32)
            dmaS(cst[:, 0:384], AP(cf_d, 0, [[2884, 128], [1, 384]]))
            ident = cst[:, 128:256]
            ptb = cpool.tile([128, 1152], BF16)
            dmaS(ptb[:], cb_d[:])
            identb = ptb[:, 1024:1152]
            # PE warm-up: dense bf16 matmuls so the HAM un-throttles
            # before the fp32 phases start
            warm_ps = psA.tile([128, 512], F32, tag="psA", name="warm_ps")
            for wi in range(14):
                nc.tensor.matmul(warm_ps[:], identb,
                                 ptb[:, 0:512],
                                 start=True, stop=True)
            cstu = cpool.tile([128, 64], U16)
            dmaA(cstu[:], cu_d[:])
            ixX = cstu[:, 0:64]

            ktl = wpool.tile([128, 256], F32)  # K 4 col-tiles
            dmaS(ktl[:], AP(k_d, 0, [[64, 128], [8192, 4], [1, 64]]))
            v_sb = wpool.tile([128, 256], F32)
            dmaA(v_sb[:], AP(v_d, 0, [[64, 128], [8192, 4], [1, 64]]))

            # data_x base rows (needed by conv1): one overlapping-window
            # fanout DMA; rows 0:12 = (ci, k1x)-shifted padded images
            data_x = wpool.tile([128, 2888], F32)
            dxp = data_x.ap[0][0]
            nc.vector.memset(data_x[0:16, :], 0.0)
            dmaS(data_x[0:12, :],
                 v(xp2, 0, [[xpitch, 3], [1, 4], [1, 2888]]))

            # conv1 stationary w1kx [12 rows=(ci,k1x), 128 free=(m,k1y)]:
            # DVE reorder -> shift-replica fanout -> PE transpose
            w1r = wpool.tile([32, 48], F32)    # free (k1y, ci, k1x)
            nc.vector.tensor_copy(
                v(w1r, 0, [[w1r.ap[0][0], 32], [12, 4], [4, 3], [1, 4]]),
                v(w1sb, 0, [[w1sb.ap[0][0], 32], [4, 4], [16, 3], [1, 4]]))
            w1kxT = wpool.tile([128, 12], F32)  # rows (m,k1y)
            dmaA(w1kxT[:],
                 v(w1r, 0, [[w1r.ap[0][0], 32], [12, 4], [1, 12]]))
            w1kx_ps = psB.tile([12, 128], F32, tag="psB", name="w1kx_ps")
            nc.tensor.transpose(w1kx_ps[:], w1kxT[:], ident)
            w1kx = wpool.tile([12, 128], F32)
            nc.scalar.copy(w1kx[:], w1kx_ps[:])

            # w2cxT [64, 512]: free (k2y, m, k2x) reorder of w2c2
            w2cxT = wpool.tile([64, 512], F32)
            nc.vector.tensor_copy(
                v(w2cxT, 0,
                  [[w2cxT.ap[0][0], 64], [128, 4], [4, 32], [1, 4]]),
                v(w2c2, 0,
                  [[w2c2.ap[0][0], 64], [4, 4], [16, 32], [1, 4]]))
            # conv2 stationary w2x [128 rows=(m,k2x), 256]: per-half
            # fanout + transpose; half h free = (c, k2y%2)
            w2x = wpool.tile([128, 256], F32)
            w2xp = w2x.ap[0][0]
            for h2 in range(2):
                wxT = wpool.tile([128, 128], F32, name=f"wxT{h2}")
                dmaA(wxT[:],
                     v(w2cxT, h2 * 256,
                       [[w2cxT.ap[0][0], 64], [128, 2], [1, 128]]))
                wx_ps = psB.tile([128, 128], F32, tag="psB",
                                 name=f"wx_ps{h2}")
                nc.tensor.transpose(wx_ps[:], wxT[:], ident)
                nc.scalar.copy(w2x[:, h2 * 128:(h2 + 1) * 128], wx_ps[:])

            # ---- Phase A: conv1 (4 tap-merged matmuls) ----
            y1ps = psA.tile([32, 512], F32, tag="psA", name="y1ps")
            for k1y in range(4):
                nc.tensor.matmul(
                    y1ps[:],
                    v(w1kx, k1y, [[w1kx.ap[0][0], 12], [4, 32]]),
                    v(data_x, 78 + 38 * k1y,
                      [[dxp, 12], [1444, 2], [76, 16], [2, 16]]),
                    start=(k1y == 0), stop=(k1y == 3))
            y1sb = wpool.tile([32, 512], F32)  # [m, (b,py,px)]
            nc.scalar.activation(y1sb[:], y1ps[:], AF.Relu, bias=b1t[:])

            # ---- x-side staging (overlaps A/B; off critical path) ----
            # w1sb2p / w1fp build
            w1sb2 = wpool.tile([32, 48], F32)   # [m, (k1x,ci,k1y)]
            dmaA(w1sb2[:], AP(w1_d, 0, [[48, 32], [16, 3], [1, 16]]))
            w1sb2p = wpool.tile([32, 64], F32)  # [m, (k1y,ci4,k1x)]
            nc.vector.memset(w1sb2p[:], 0.0)
            nc.vector.tensor_copy(
                v(w1sb2p, 0, [[w1sb2p.ap[0][0], 32], [16, 4], [4, 3], [1, 4]]),
                v(w1sb2, 0, [[w1sb2.ap[0][0], 32], [4, 4], [16, 3], [1, 4]]))
            w1fp_ps = psB.tile([64, 32], F32, tag="psB", name="w1fp_ps")
            nc.tensor.transpose(w1fp_ps[:], w1sb2p[:], cst[0:32, 128:160])
            w1fp = wpool.tile([128, 32], BF16)
            nc.scalar.copy(w1fp[0:64, :], w1fp_ps[:])
            dmaS(w1fp[64:128, :], w1fp[0:64, :])

            # data_x replicas for the x-side gather
            for d in (16, 32, 64):
                dmaA(data_x[d:2 * d, :], data_x[0:d, :])
            xgall = wpool.tile([128, 1024], F32)
            nc.gpsimd.indirect_copy(
                v(xgall, 0, [[xgall.ap[0][0], 128], [1, 1024], [1, 1]]),
                data_x[:], ixX, True)

            # K^T via 4 PE transposes (f32 for h1, bf16 copy for h2)
            kt_sb = wpool.tile([64, 512], F32)
            kt_b = wpool.tile([64, 512], BF16)
            for t in range(4):
                kt_ps = psB.tile([64, 128], F32, tag="psB", name="kt_ps")
                nc.tensor.transpose(kt_ps[:], ktl[:, t * 64:(t + 1) * 64],
                                    ident)
                nc.scalar.copy(kt_sb[:, t * 128:(t + 1) * 128], kt_ps[:])
                nc.vector.tensor_copy(kt_b[:, t * 128:(t + 1) * 128],
                                      kt_ps[:])
            v_b = wpool.tile([128, 256], BF16)
            nc.vector.tensor_copy(v_b[:], v_sb[:])

            # ---- Phase A: conv2 (4 tap-merged matmuls) ----
            y1p = wpool.tile([32, 652], F32)   # [m, (b,18,18)] padded
            nc.vector.memset(y1p[:], 0.0)
            ypitch = y1p.ap[0][0]
            nc.vector.tensor_copy(
                v(y1p, 19, [[ypitch, 32], [324, 2], [18, 16], [1, 16]]),
                v(y1sb, 0,
                  [[y1sb.ap[0][0], 32], [256, 2], [16, 16], [1, 16]]))
            m1p = wpool.tile([32, 652], F32)
            nc.vector.tensor_scalar(m1p[:], y1p[:], 0.0, None, ALU.is_gt)

            def tapview(tl, k2, pitch):
                k2y, k2x = k2 // 4, k2 % 4
                return v(tl, 18 * k2y + k2x,
                         [[pitch, 32], [324, 2], [36, 8], [2, 8]])

            # y1x [128 rows=(m,k2x), 648]: k2x-shift replicas of y1p
            y1x = wpool.tile([128, 648], F32)
            yxp = y1x.ap[0][0]
            dmaS(y1x[:], v(y1p, 0, [[ypitch, 32], [1, 4], [1, 648]]))

            ypre = psA.tile([64, 128], F32, tag="psA", name="ypre")
            for k2y in range(4):
                nc.tensor.matmul(
                    ypre[:],
                    v(w2x, (k2y // 2) * 128 + k2y % 2,
                      [[w2xp, 128], [2, 64]]),
                    v(y1x, 18 * k2y,
                      [[yxp, 128], [324, 2], [36, 8], [2, 8]]),
                    start=(k2y == 0), stop=(k2y == 3))
            yT = wpool.tile([64, 128], F32)    # [c, (b,o)]
            nc.scalar.activation(yT[:], ypre[:], AF.Relu, bias=b2t[:])
            m2T = wpool.tile([64, 128], F32)
            nc.vector.tensor_scalar(m2T[:], yT[:], 0.0, None, ALU.is_gt)

            # m1x [128 rows=(m,k2x), 648]: k2x-shift replicas of m1p
            m1x = wpool.tile([128, 648], F32)
            mxp = m1x.ap[0][0]
            dmaA(m1x[:], v(m1p, 0, [[m1p.ap[0][0], 32], [1, 4], [1, 648]]))

            def hopfield(src, tag, kt, vv, idn, pdt):
                """src [c 64, (b,o) 128] -> yq [(b,o) 128, c 64] SBUF f32."""
                a_ps = psA.tile([128, 512], F32, tag="psA",
                                name=f"a_ps{tag}")
                nc.tensor.matmul(a_ps[:], src, kt, start=True, stop=True)
                rmax = wpool.tile([128, 1], F32, name=f"rmax{tag}")
                nc.vector.tensor_reduce(rmax[:], a_ps[:], AX.X, ALU.max)
                negbm = wpool.tile([128, 1], F32, name=f"negbm{tag}")
                nc.vector.tensor_scalar(negbm[:], rmax[:], -0.125, None,
                                        ALU.mult)
                p_sb = wpool.tile([128, 512], pdt, name=f"p_sb{tag}")
                ssum = wpool.tile([128, 1], F32, name=f"ssum{tag}")
                nc.scalar.activation(p_sb[:], a_ps[:], AF.Exp, bias=negbm[:],
                                     scale=0.125, accum_out=ssum[:])
                rec = wpool.tile([128, 1], F32, name=f"rec{tag}")
                nc.vector.reciprocal(rec[:], ssum[:])
                yq_ps = psC.tile([128, 64], F32, tag="psC",
                                 name=f"yq_ps{tag}")
                for t in range(4):
                    pt_ps = psB.tile([128, 128], pdt, tag="psB",
                                     name=f"pt_ps{tag}{t}")
                    nc.tensor.transpose(pt_ps[:],
                                        p_sb[:, t * 128:(t + 1) * 128],
                                        idn)
                    pt_sb = wpool.tile([128, 128], pdt, tag="pt_sb",
                                       name=f"pt_sb{tag}{t}")
                    nc.scalar.copy(pt_sb[:], pt_ps[:])
                    nc.tensor.matmul(yq_ps[:], pt_sb[:],
                                     vv[:, t * 64:(t + 1) * 64],
                                     start=(t == 0), stop=(t == 3))
                yq_sb = wpool.tile([128, 64], F32, name=f"yq_sb{tag}")
                nc.scalar.activation(yq_sb[:], yq_ps[:], AF.Copy,
                                     scale=rec[:])
                return yq_sb

            dmaA(cst[:, 384:2884],
                 AP(cf_d, 384, [[2884, 128], [1, 2500]]))
            yq1 = hopfield(yT[:], "h1", kt_sb[:], v_sb, ident, F32)

            yqT_ps = psB.tile([64, 128], F32, tag="psB", name="yqT_ps")
            nc.tensor.transpose(yqT_ps[:], yq1[:], ident)
            r2T = wpool.tile([64, 128], F32)
            nc.vector.scalar_tensor_tensor(r2T[:], yqT_ps[:], -1.0, yT[:],
                                           ALU.mult, ALU.add)
            nc.vector.tensor_mul(r2T[:], r2T[:], m2T[:])

            # ---- Phase B: e_patch + argmin ----
            w1s16 = wpool.tile([32, 16], F32)
            nc.vector.tensor_reduce(
                w1s16[:],
                v(w1sb, 0, [[w1sb.ap[0][0], 32], [1, 16], [16, 3]]),
                AX.X, ALU.add)
            w1si = wpool.tile([32, 256], F32)  # w1s 16x16 zero-pad image
            nc.vector.memset(w1si[:], 0.0)
            nc.vector.tensor_copy(
                v(w1si, 102, [[w1si.ap[0][0], 32], [16, 4], [1, 4]]),
                w1s16[:])
            # w1si4 [128 rows=(m,k2x), 250]: -2*k2x-shifted w1si replicas
            w1si4 = wpool.tile([128, 250], F32)
            w4p = w1si4.ap[0][0]
            for k2x in range(4):
                (dmaS if k2x % 2 == 0 else dmaA)(
                    v(w1si4, k2x * w4p, [[4 * w4p, 32], [1, 250]]),
                    v(w1si, 6 - 2 * k2x, [[w1si.ap[0][0], 32], [1, 250]]))
            # w1stpX [128, 400]: per-k2y 10x10 windows
            w1stpX = wpool.tile([128, 400], F32)
            for k2y in range(4):
                nc.vector.tensor_copy(
                    v(w1stpX, k2y * 100,
                      [[w1stpX.ap[0][0], 128], [10, 10], [1, 10]]),
                    v(w1si4, (6 - 2 * k2y) * 16,
                      [[w4p, 128], [16, 10], [1, 10]]))

            # G + mask -> gm [128 rows=(m,k2x), 512 (k2y, (b,o))]
            gm = wpool.tile([128, 512], F32)
            for k2y in range(4):
                g_ps = psB.tile([128, 128], F32, tag="psB",
                                name=f"g_ps{k2y}")
                nc.tensor.matmul(
                    g_ps[:],
                    w2cxT[:, k2y * 128:(k2y + 1) * 128],
                    r2T[:], start=True, stop=True)
                nc.vector.tensor_tensor(
                    gm[:, k2y * 128:(k2y + 1) * 128], g_ps[:],
                    v(m1x, 18 * k2y,
                      [[mxp, 128], [324, 2], [36, 8], [2, 8]]),
                    ALU.mult)

            ep_ps = psA.tile([100, 128], F32, tag="psA", name="ep_ps")
            for k2y in range(4):
                nc.tensor.matmul(
                    ep_ps[:],
                    w1stpX[:, k2y * 100:(k2y + 1) * 100],
                    gm[:, k2y * 128:(k2y + 1) * 128],
                    start=(k2y == 0), stop=(k2y == 3))
            ep_sb = wpool.tile([100, 128], F32)
            nc.scalar.copy(ep_sb[:], ep_ps[:])
            ep2_ps = psB.tile([128, 100], F32, tag="psB", name="ep2_ps")
            nc.tensor.transpose(ep2_ps[:], ep_sb[:], cst[0:100, 128:228])
            # zero-padded ep2 for shifted candidate reads
            ep2p = wpool.tile([128, 300], F32)
            nc.vector.memset(ep2p[:], 0.0)
            nc.scalar.copy(ep2p[:, 100:200], ep2_ps[:])
            e2p = ep2p.ap[0][0]

            # 25-candidate stack: shift matmuls into PSUM (2 rounds of
            # <=13 slots, 3-bank pool), each followed by one DVE add of
            # the validity offsets
            cands = [(a, c) for a in range(-2, 3) for c in range(-2, 3)]
            stk = wpool.tile([128, 2500], F32)
            cp = cst.ap[0][0]
            for half, (j0, j1) in enumerate(((0, 13), (13, 25))):
                n = j1 - j0
                sps = psD.tile([128, n * 100], F32, tag="psD",
                               name=f"stk_ps{half}")
                for j in range(j0, j1):
                    a, c = cands[j]
                    dpp = 8 * a + c
                    dff = -40 * a - 4 * c
                    nc.tensor.matmul(
                        sps[:, (j - j0) * 100:(j - j0 + 1) * 100],
                        v(cst, 128 + dpp, [[cp, 128], [1, 128]]),
                        v(ep2p, 100 + dff, [[e2p, 128], [1, 100]]),
                        start=True, stop=True)
                nc.vector.tensor_tensor(
                    stk[:, j0 * 100:j1 * 100], sps[:],
                    cst[:, 384 + j0 * 100:384 + j1 * 100], ALU.add)

            vmin = wpool.tile([128, 100], F32)
            skp = stk.ap[0][0]
            nc.vector.tensor_reduce(
                vmin[:], v(stk, 0, [[skp, 128], [1, 100], [100, 25]]),
                AX.X, ALU.min)
            emin = wpool.tile([128, 100], F32)
            nc.vector.tensor_reduce(
                emin[:], v(stk, 0, [[skp, 128], [1, 100], [100, 12]]),
                AX.X, ALU.min)
            # msk = (self == vmin) & (vmin < 0) & (emin > vmin)
            weq = wpool.tile([128, 100], F32)
            nc.vector.tensor_tensor(weq[:], stk[:, 1200:1300], vmin[:],
                                    ALU.is_equal)
            wgt = wpool.tile([128, 100], F32)
            nc.vector.tensor_tensor(wgt[:], emin[:], vmin[:], ALU.is_gt)
            wneg = wpool.tile([128, 100], F32)
            nc.vector.tensor_scalar(wneg[:], vmin[:], 0.0, None, ALU.is_lt)
            msk = wpool.tile([128, 100], F32)
            nc.vector.tensor_tensor(msk[:], weq[:], wneg[:], ALU.mult)
            nc.vector.tensor_tensor(msk[:], msk[:], wgt[:], ALU.mult)

            # transpose to patch-major + bf16 for the expansion matmuls
            mskT_ps = psB.tile([100, 128], F32, tag="psB", name="mskT_ps")
            nc.tensor.transpose(mskT_ps[:], msk[:], ident)
            mskTb = wpool.tile([100, 128], BF16)
            nc.scalar.copy(mskTb[:], mskT_ps[:])

            # ---- Phase C: mask expansion + masked conv ----
            xsel = []
            for t in range(8):
                mx_ps = psB.tile([128, 128], F32, tag="psB", name=f"mx{t}")
                nc.tensor.matmul(mx_ps[:],
                                 ptb[0:100, t * 128:(t + 1) * 128],
                                 mskTb[:], start=True, stop=True)
                xs = wpool.tile([128, 128], BF16, name=f"xs{t}")
                nc.vector.tensor_tensor(xs[:],
                                        xgall[:, t * 128:(t + 1) * 128],
                                        mx_ps[:], ALU.mult)
                xsel.append(xs)

            zm = [wpool.tile([32, 128], BF16, name=f"zm{k2}")
                  for k2 in range(16)]
            for k2 in range(16):
                k2y, k2x = k2 // 4, k2 % 4
                t = k2y * 2 + k2x // 2
                half = (k2x % 2) * 64
                z_ps = psB.tile([32, 128], F32, tag="psB", name=f"z_ps{k2}")
                nc.tensor.matmul(z_ps[:], w1fp[half:half + 64, :],
                                 xsel[t][half:half + 64, :],
                                 start=True, stop=True)
                nc.vector.tensor_tensor(zm[k2][:], z_ps[:],
                                        tapview(m1p, k2, ypitch), ALU.mult)

            ym_ps = psA.tile([128, 64], F32, tag="psA", name="ym_ps")
            for k2 in range(16):
                nc.tensor.matmul(
                    ym_ps[:], zm[k2][:],
                    v(w2sb, k2, [[w2sb.ap[0][0], 32], [16, 64]]),
                    start=(k2 == 0), stop=(k2 == 15))

            yTT_ps = psB.tile([128, 64], F32, tag="psB", name="yTT_ps")
            nc.tensor.transpose(yTT_ps[:], yT[:], cst[0:64, 128:192])
            m2g = wpool.tile([128, 64], F32)
            nc.vector.tensor_scalar(m2g[:], yTT_ps[:], 0.0, None, ALU.is_gt)
            ymm = wpool.tile([128, 64], BF16)
            nc.vector.tensor_tensor(ymm[:], ym_ps[:], m2g[:], ALU.mult)

            t2_ps = psB.tile([64, 128], BF16, tag="psB", name="t2_ps")
            nc.tensor.transpose(t2_ps[:], ymm[:], identb)
            ymmT = wpool.tile([64, 128], BF16)
            nc.scalar.copy(ymmT[:], t2_ps[:])

            yq2 = hopfield(ymmT[:], "h2", kt_b[:], v_b, identb, BF16)

            tr_ps = psB.tile([64, 128], F32, tag="psB", name="tr_ps")
            nc.tensor.transpose(tr_ps[:], yq2[:], ident)
            outT = wpool.tile([64, 128], F32)
            nc.scalar.copy(outT[:], tr_ps[:])
            for b in range(2):
                (dmaS if b == 0 else dmaA)(
                    AP(out_d, b * 4096, [[64, 64], [8, 8], [1, 8]]),
                    outT[:, b * 64:(b + 1) * 64])

    return nc


_CACHE = {}


def kernel(**inputs) -> np.ndarray:
    from concourse.bass_utils import run_bass_kernel_spmd
    if "nc" not in _CACHE:
        from concourse import bacc
        nc = bacc.Bacc("TRN2", target_bir_lowering=False, debug=False,
                       num_devices=N_CORES)
        build_program(nc)
        nc.compile()
        _CACHE["nc"] = nc
        _CACHE["consts"] = _consts()
    nc = _CACHE["nc"]
    feed = {}
    for k, val in inputs.items():
        arr = np.asarray(val)
        if arr.dtype != np.uint16:
            arr = np.asarray(arr, np.float32)
        feed[k] = np.ascontiguousarray(arr)
    for k, val in _CACHE["consts"].items():
        feed[k] = val
    in_maps = [dict(feed) for _ in range(N_CORES)]
    res = run_bass_kernel_spmd(nc, in_maps, list(range(N_CORES)))
    return np.asarray(res.results[0]["out"], np.float32)
